# revision 5
# baseline (speedup 1.0000x reference)
"""Multi-head attention (nonstandard softmax normalization) on 8 Trainium2
cores.

Sharding: core c -> (batch c//2, head-group c%2 of 8 heads); each core runs
its 8 heads end-to-end plus the partial Wo product over its 512 feature rows;
the host sums the two partial products per batch.

Per-core design (S=1024, 4 head-pairs, fp16 matmuls):
 - ACT-paced software pipeline: the 64 [128,1024] exps on the scalar engine
   (~64us) and the PE work stream are interleaved so neither starves; PE fill
   work (AV of the previous pair, projections of the next pair, vT chunks)
   slots between QK psum generations.
 - QK runs the two heads of a pair as row-co-executing K=64 matmuls
   (lone K=64 matmuls stream at ~2 cycles/col; the row-disjoint pair
   restores ~1 cycle/col aggregate).
 - denom[t] = sum_u e[t,u] via 4-way column-tiled ones-matmuls (M=32
   replication -> all psum rows valid); per-head accumulation streaks are
   kept disjoint in time (interleaved open groups in one psum bank corrupt).
   A DRAM roundtrip transposes the free-dim denom to partitions; recip is
   folded into vT before the AV matmuls.
 - AV co-executes the two heads as column-tiled M=64 matmuls.
 - tail: while the last pair's denom roundtrip is in flight, 5 Wo tiles
   open partial fc0..2 accumulations in psum banks freed by the QK pools;
   only their fc3 contribution remains after the last AV.
 - psum budget (8 banks): QK gens 2x[128,1024], denom 1, AV 1, shared 2.
"""

import os
import sys
import types

import numpy as np

import concourse.bass as bass
import concourse.mybir as mybir
import concourse.tile as tile
from contextlib import ExitStack
from collections import deque

_f32 = mybir.dt.float32
_f16 = mybir.dt.float16


def _install_ntff_shim():
    """Register the axon NTFF profile hook if the image's antenv lacks it.

    Lets run_bass_kernel_spmd(trace=True) return exec_time_ns. Harmless if
    already present.
    """
    try:
        import antenv.axon_hooks  # noqa: F401
        return
    except ImportError:
        pass
    try:
        import antenv
        from trn_agent_boot.trn_boot import _ntff_profile_via_ctypes
    except ImportError:
        return
    mod = types.ModuleType("antenv.axon_hooks")
    mod._hook = None

    def set_axon_ntff_profile_hook(h):
        mod._hook = h

    def get_axon_ntff_profile_hook():
        return mod._hook

    mod.set_axon_ntff_profile_hook = set_axon_ntff_profile_hook
    mod.get_axon_ntff_profile_hook = get_axon_ntff_profile_hook
    sys.modules["antenv.axon_hooks"] = mod
    antenv.axon_hooks = mod
    for so in ("/opt/axon/libaxon_pjrt.so",):
        if os.path.exists(so):
            try:
                mod._hook = _ntff_profile_via_ctypes(so)
            except Exception:
                mod._hook = None
            break


def _install_drain_patch():
    """Work around this toolchain's walrus rejecting sem waits on Drain.

    TileContext's final drain carries end-of-kernel semaphore waits inline;
    this walrus build encodes Drain as NEURON_ISA_TPB_CTRL_NO_STRUCT and
    fails codegen ("Too many sync wait commands") for ANY inline wait.
    Equivalent semantics: emit the waits as standalone sync-engine wait
    instructions and leave the Drain bare.
    """
    if getattr(tile.TileContext, "_drain_patch_installed", False):
        return
    from concourse.vector_clock import ScopedClock

    def _patched_drain_and_barrier(self, tick_clock, wait_clock):
        drain_inst = self.nc.sync.drain()
        wait_clock.add_sem_waits(
            drain_inst.ins, ScopedClock({None: tick_clock.global_clock})
        )
        si = drain_inst.ins.sync_info
        waits = list(si.on_wait) if si is not None else []
        if waits:
            drain_inst.ins.sync_info = mybir.SyncInfo(
                on_wait=[], on_update=list(si.on_update) if si.on_update else []
            )
            by_name = (
                {h.name: h for h in self.sems.allocated().values()}
                if self.sems is not None else {}
            )
            for w in waits:
                sem = by_name.get(w.ant_name)
                assert sem is not None, f"unknown drain-wait sem: {w.ant_name}"
                assert w.wait_mode == "sem-ge-imm", w
                self.nc.sync.wait_ge(sem, w.wait_value)
        self.nc.all_engine_barrier()
        assert self.sems is not None
        popped = self.nc._tile_sem_poison_stack.pop()
        assert popped is self._sem_poison
        self.nc.clear_and_free_semaphores(list(self.sems.allocated().values()))
        self.nc.all_engine_barrier()

    tile.TileContext._drain_and_barrier = _patched_drain_and_barrier

    # Same walrus limitation, general form: at most ONE inline sem wait per
    # instruction. Tile's wait assignment can attach several (e.g. a DMA
    # waiting on a slot freed by PE + DVE + another queue). Hoist all but
    # the last wait onto same-engine EventSemaphore carrier instructions.
    orig_add = tile.TileContext._add_instruction

    def _split_add_instruction(self, inst):
        si = inst.sync_info
        if si is not None and si.on_wait and len(si.on_wait) > 1:
            waits = list(si.on_wait)
            for w in waits[:-1]:
                ev = mybir.InstEventSemaphore(
                    name=self.nc.get_next_instruction_name(),
                    engine=inst.engine,
                    sync_info=mybir.SyncInfo(on_wait=[w], on_update=[]),
                )
                orig_add(self, ev)
            inst.sync_info = mybir.SyncInfo(
                on_wait=[waits[-1]],
                on_update=list(si.on_update) if si.on_update else [],
            )
        orig_add(self, inst)

    tile.TileContext._add_instruction = _split_add_instruction
    tile.TileContext._drain_patch_installed = True


def build_core_kernel_v2(S=1024, n_pairs=4, e_out=1024, debug=False):
    _install_drain_patch()

    C = S // 128          # 8 t-chunks per pair
    NT = 512              # matmul moving tile (= psum bank)
    NS = S // NT          # 2
    S4 = S // 4           # 256 denom col-group width
    EC = e_out // 128     # 8 output row-chunks
    FP = n_pairs * 128    # 512 feature rows on this core

    nc = bass.Bass()
    q_rows = nc.declare_dram_parameter("q_rows", [FP, S], _f16, isOutput=False)
    k_rows = nc.declare_dram_parameter("k_rows", [FP, S], _f16, isOutput=False)
    v_rows = nc.declare_dram_parameter("v_rows", [FP, S], _f16, isOutput=False)
    # host-side pre-arranged: w{q,k,v}T[p, pr, m] (blockdiag pair weights,
    # partition-major) and woT[p, pr, e] = Wo.T[pr*128+p, e]
    wqT = nc.declare_dram_parameter("wqT", [128, n_pairs, 128], _f16, isOutput=False)
    wkT = nc.declare_dram_parameter("wkT", [128, n_pairs, 128], _f16, isOutput=False)
    wvT = nc.declare_dram_parameter("wvT", [128, n_pairs, 128], _f16, isOutput=False)
    woT = nc.declare_dram_parameter("woT", [128, n_pairs, e_out], _f16, isOutput=False)
    out_part = nc.declare_dram_parameter("out_part", [e_out, S], _f32, isOutput=True)
    if debug:
        dbg_rc = nc.declare_dram_parameter("dbg_rc", [n_pairs, 128, 2, 8], _f32, isOutput=True)
        dbg_E = nc.declare_dram_parameter("dbg_E", [128, 1024], _f16, isOutput=True)
        dbg_vt = nc.declare_dram_parameter("dbg_vt", [n_pairs, 128, 8, 128], _f16, isOutput=True)
        dbg_dstage = nc.declare_dram_parameter("dbg_dstage", [n_pairs, 128, 512], _f32, isOutput=True)

    Exp = mybir.ActivationFunctionType.Exp
    Mult = mybir.AluOpType.mult

    with tile.TileContext(nc) as tc, ExitStack() as ctx:
        consts = ctx.enter_context(tc.tile_pool(name="consts", bufs=1))
        wop = ctx.enter_context(tc.tile_pool(name="wop", bufs=1))
        raws = ctx.enter_context(tc.tile_pool(name="raws", bufs=2))
        qks = ctx.enter_context(tc.tile_pool(name="qks", bufs=2))
        outp = ctx.enter_context(tc.tile_pool(name="outp", bufs=1))
        vts = ctx.enter_context(tc.tile_pool(name="vts", bufs=n_pairs))
        Ep = ctx.enter_context(tc.tile_pool(name="Ep", bufs=4))
        dstp = ctx.enter_context(tc.tile_pool(name="dstp", bufs=2))
        rcp = ctx.enter_context(tc.tile_pool(name="rcp", bufs=2))
        wostp = ctx.enter_context(tc.tile_pool(name="wostp", bufs=3))
        dram = ctx.enter_context(tc.tile_pool(name="dscratch", bufs=2, space="DRAM"))
        # psum: allocation order fixes bank layout; exactly 8 banks
        pqk = ctx.enter_context(tc.tile_pool(name="pqk", bufs=2, space="PSUM"))
        dpsp = ctx.enter_context(tc.tile_pool(name="dpsp", bufs=1, space="PSUM"))
        avpp = ctx.enter_context(tc.tile_pool(name="avpp", bufs=1, space="PSUM"))
        psm = ctx.enter_context(tc.tile_pool(name="psm", bufs=2, space="PSUM"))

        ones32 = consts.tile([128, 32], _f16, tag="ones32")
        nc.vector.memset(ones32, 1.0)
        dummy = consts.tile([128, NT], _f16, tag="dummy")
        nc.vector.memset(dummy, 0.0)
        wq_sb = consts.tile([128, n_pairs, 128], _f16, tag="wq")
        wk_sb = consts.tile([128, n_pairs, 128], _f16, tag="wk")
        wv_sb = consts.tile([128, n_pairs, 128], _f16, tag="wv")

        # ---- input DMAs for pair 0 first, then weights, then the rest ----
        qr_t, kr_t, vr_t = {}, {}, {}

        def issue_raws(pr):
            qr = raws.tile([128, S], _f16, tag="qr", name=f"qr{pr}")
            kr = raws.tile([128, S], _f16, tag="kr", name=f"kr{pr}")
            vr = raws.tile([128, S], _f16, tag="vr", name=f"vr{pr}")
            qr_t[pr], kr_t[pr], vr_t[pr] = qr, kr, vr
            rs = slice(pr * 128, (pr + 1) * 128)
            nc.sync.dma_start(out=qr, in_=q_rows[rs, :])
            nc.sync.dma_start(out=kr, in_=k_rows[rs, :])
            nc.sync.dma_start(out=vr, in_=v_rows[rs, :])

        issue_raws(0)
        nc.sync.dma_start(out=wq_sb, in_=wqT[:, :, :])
        nc.sync.dma_start(out=wk_sb, in_=wkT[:, :, :])
        nc.sync.dma_start(out=wv_sb, in_=wvT[:, :, :])
        issue_raws(1)
        woT_sb = wop.tile([128, n_pairs, e_out], _f16, tag="woT")
        nc.sync.dma_start(out=woT_sb, in_=woT[:, :, :])

        # ---- PE warm-up during the DMA lead-in ----
        for _ in range(4):
            ps = psm.tile([128, NT], _f32, tag="ps")
            nc.tensor.matmul(ps[0:32, :], lhsT=ones32, rhs=dummy,
                             start=True, stop=True)

        q_all_t, k_all_t, vt_t, E_t = {}, {}, {}, {}
        out_all = outp.tile([128, n_pairs, S], _f16, tag="outall")

        # ---------- emission helpers ----------
        def proj_unit(pr, which, st):
            """One [128,512] projection matmul + copy into q_all/k_all."""
            if which == "q":
                src, wt = qr_t[pr], wq_sb
                if pr not in q_all_t:
                    q_all_t[pr] = qks.tile([128, S], _f16, tag="qa", name=f"qa{pr}")
                dst = q_all_t[pr]
            else:
                src, wt = kr_t[pr], wk_sb
                if pr not in k_all_t:
                    k_all_t[pr] = qks.tile([128, S], _f16, tag="ka", name=f"ka{pr}")
                dst = k_all_t[pr]
            ps = psm.tile([128, NT], _f32, tag="ps")
            nc.tensor.matmul(ps, lhsT=wt[:, pr, :],
                             rhs=src[:, st * NT:(st + 1) * NT],
                             start=True, stop=True)
            nc.vector.tensor_copy(out=dst[:, st * NT:(st + 1) * NT], in_=ps)

        def vt_unit(pr, c):
            """vT chunk c: [128t,128i] via transpose-projection matmul."""
            if pr not in vt_t:
                vt_t[pr] = vts.tile([128, C, 128], _f16, tag="vt", name=f"vt{pr}")
            vrc = vr_t[pr].rearrange("p (c t) -> p c t", c=C)
            ps = psm.tile([128, NT], _f32, tag="ps")
            nc.tensor.matmul(ps[:, 0:128], lhsT=vrc[:, c, :],
                             rhs=wv_sb[:, pr, :], start=True, stop=True)
            nc.vector.tensor_copy(out=vt_t[pr][:, c, :], in_=ps[:, 0:128])

        def qk_pair_gen(pr, c):
            """QK chunk c, both heads row-co-executing, + the two exps.

            K=64 matmuls run ~2 cycles/col alone; interleaving the two heads
            (disjoint PE row halves) restores ~1 cycle/col aggregate. Order
            (h0,st0),(h1,st0),(h1,st1),(h0,st1) chains same-stationary pairs.
            """
            ka = k_all_t[pr]
            qa = q_all_t[pr]
            pst = {hh: pqk.tile([128, S], _f32, tag="pqk",
                                name=f"pqk_{pr}_{c}_{hh}") for hh in (0, 1)}

            def mm(hh, st):
                nc.tensor.matmul(
                    pst[hh][:, st * NT:(st + 1) * NT],
                    lhsT=ka[64 * hh:64 * hh + 64, c * 128:(c + 1) * 128],
                    rhs=qa[64 * hh:64 * hh + 64, st * NT:(st + 1) * NT],
                    start=True, stop=True)

            mm(0, 0)
            mm(1, 0)
            mm(1, 1)
            mm(0, 1)
            for hh in (0, 1):
                nc.scalar.activation(out=E_t[(pr, hh)][:, c, :], in_=pst[hh],
                                     func=Exp, scale=0.125)

        dps_t = {}

        def denom_gen(pr, hh, c):
            """4-way col-tiled ones-matmuls (fast co-exec); h0/h1 share the
            bank col-split, so the two heads' groups must NOT interleave in
            time (emit h0's full c-streak, then h1's)."""
            if pr not in dps_t:
                dps_t[pr] = dpsp.tile([128, NT], _f32, tag="dps", name=f"dps{pr}")
            dps = dps_t[pr]
            E = E_t[(pr, hh)]
            for q4 in range(4):
                nc.tensor.matmul(
                    dps[32 * q4:32 * q4 + 32, 256 * hh:256 * hh + 256],
                    lhsT=ones32,
                    rhs=E[:, c, q4 * S4:(q4 + 1) * S4],
                    start=(c == 0), stop=(c == C - 1),
                    tile_position=(0, 32 * q4),
                    skip_group_check=True)

        def roundtrip_scale(pr):
            """denom psum -> DRAM transpose -> recip -> fold into vT."""
            dstage = dstp.tile([128, 2, 256], _f32, tag="dst", name=f"dst{pr}")
            nc.vector.tensor_copy(out=dstage, in_=dps_t[pr].rearrange(
                "p (h x) -> p h x", h=2))
            scr = dram.tile([2048], _f32, tag="scr", name=f"scr{pr}")
            # scr[h*1024 + q4*256 + x] = denom_h[q4*256 + x]
            nc.sync.dma_start(
                out=scr.rearrange("(h a f) -> a h f", h=2, a=4),
                in_=dstage[0:97:32, :, :])
            rcr = rcp.tile([128, 2, C], _f32, tag="rcr", name=f"rcr{pr}")
            rc = rcp.tile([128, 2, C], _f32, tag="rc", name=f"rc{pr}")
            nc.sync.dma_start(
                out=rcr,
                in_=scr.rearrange("(h c p) -> p h c", h=2, p=128))
            nc.vector.reciprocal(out=rc, in_=rcr)
            vt = vt_t[pr]
            for hh in (0, 1):
                nc.vector.tensor_tensor(
                    out=vt[:, :, 64 * hh:64 * hh + 64],
                    in0=vt[:, :, 64 * hh:64 * hh + 64],
                    in1=rc[:, hh, :, None].to_broadcast((128, C, 64)),
                    op=Mult)
            if debug:
                nc.sync.dma_start(out=dbg_rc[pr], in_=rc)
                nc.sync.dma_start(out=dbg_dstage[pr], in_=dstage)
                nc.sync.dma_start(out=dbg_vt[pr], in_=vt)
                if pr == 0:
                    nc.sync.dma_start(out=dbg_E[:, :], in_=E_t[(0, 0)][:, 0, :])

        av_t = {}

        def av_unit(pr, st, c0, nch=2):
            """AV chunks [c0, c0+nch) chained so vt stationary loads hide
            behind the previous co-exec pair's stream."""
            if (pr, st) not in av_t:
                av_t[(pr, st)] = avpp.tile([128, NT], _f32, tag="avp",
                                           name=f"avp{pr}_{st}")
            avp = av_t[(pr, st)]
            for c in range(c0, c0 + nch):
                for hh in (0, 1):
                    nc.tensor.matmul(
                        avp[64 * hh:64 * hh + 64, :],
                        lhsT=vt_t[pr][:, c, 64 * hh:64 * hh + 64],
                        rhs=E_t[(pr, hh)][:, c, st * NT:(st + 1) * NT],
                        start=(c == 0), stop=(c == C - 1),
                        tile_position=(0, 64 * hh),
                        skip_group_check=True)

        def av_copy(pr, st):
            nc.vector.tensor_copy(
                out=out_all[:, pr, st * NT:(st + 1) * NT], in_=av_t[(pr, st)])

        # ---------- the pipeline ----------
        for pr in range(n_pairs):
            E_t[(pr, 0)] = Ep.tile([128, C, S], _f16, tag="E", name=f"E{pr}_0")
            E_t[(pr, 1)] = Ep.tile([128, C, S], _f16, tag="E", name=f"E{pr}_1")

        # initial projections for pair 0 (not hidden behind anything);
        # st0 q+k first so QK chunk 0 can start before st1 lands
        proj_unit(0, "q", 0)
        proj_unit(0, "k", 0)
        proj_unit(0, "q", 1)
        proj_unit(0, "k", 1)

        for pr in range(n_pairs):
            # fill units consumed across this pair's 16 QK gens
            # fill order matters: AV of pair p-1 depends on its denom
            # roundtrip (~3.3us latency from end of pair p-1) -- vt/proj
            # units must come first or the in-order PE queue stalls on AV
            fill = deque()
            for c in range(C):
                fill.append(lambda pr=pr, c=c: vt_unit(pr, c))
            if pr + 1 < n_pairs:
                fill.append(lambda pr=pr: proj_unit(pr + 1, "q", 0))
                fill.append(lambda pr=pr: proj_unit(pr + 1, "k", 0))
                fill.append(lambda pr=pr: proj_unit(pr + 1, "q", 1))
                fill.append(lambda pr=pr: proj_unit(pr + 1, "k", 1))
            if pr >= 1:
                for st in range(NS):
                    for c0 in range(0, C, 2):
                        fill.append(
                            lambda pr=pr, st=st, c0=c0: av_unit(pr - 1, st, c0))
                    fill.append(lambda pr=pr, st=st: av_copy(pr - 1, st))
            if pr + 2 < n_pairs:
                fill.append(lambda pr=pr: issue_raws(pr + 2))

            total = len(fill)
            done = 0
            for c in range(C):
                qk_pair_gen(pr, c)
                want = (total * (c + 1)) // C
                while done < want:
                    fill.popleft()()
                    done += 1
            while fill:
                fill.popleft()()
            for hh in (0, 1):
                for c in range(C):
                    denom_gen(pr, hh, c)
            roundtrip_scale(pr)

        # tail: while pair 3's denom roundtrip is in flight (PE would idle),
        # open partial Wo accumulations for 6 (st, ec) tiles over fc 0..2 in
        # psum banks freed by the QK/denom pools; their fc3 lands after AV p3.
        pr = n_pairs - 1

        def wo_mm(ops, fc, ec, st, start, stop):
            for mh in (0, 1):
                nc.tensor.matmul(
                    ops[64 * mh:64 * mh + 64, :],
                    lhsT=woT_sb[:, fc,
                                ec * 128 + 64 * mh:ec * 128 + 64 * mh + 64],
                    rhs=out_all[:, fc, st * NT:(st + 1) * NT],
                    start=start, stop=stop,
                    tile_position=(0, 64 * mh),
                    skip_group_check=True)

        part_tiles = {}
        holders = []
        for i in range(2):
            t = pqk.tile([128, S], _f32, tag="pqk", name=f"wopart{i}")
            holders.append(t)
        holders.append(dpsp.tile([128, NT], _f32, tag="dps", name="wopart2"))
        slots = [holders[0][:, 0:NT], holders[0][:, NT:S],
                 holders[1][:, 0:NT], holders[1][:, NT:S],
                 holders[2]]
        part_list = [(st, ec) for st in range(NS) for ec in range(3)][:5]
        for slot, (st, ec) in zip(slots, part_list):
            part_tiles[(st, ec)] = slot
            for fc in range(3):
                wo_mm(slot, fc, ec, st, start=(fc == 0), stop=False)

        for st in range(NS):
            av_unit(pr, st, 0, nch=C)
            av_copy(pr, st)

        for st in range(NS):
            for ec in range(EC):
                if (st, ec) in part_tiles:
                    ops = part_tiles[(st, ec)]
                    wo_mm(ops, n_pairs - 1, ec, st, start=False, stop=True)
                else:
                    ops = psm.tile([128, NT], _f32, tag="ps")
                    for fc in range(n_pairs):
                        wo_mm(ops, fc, ec, st,
                              start=(fc == 0), stop=(fc == n_pairs - 1))
                wost = wostp.tile([128, NT], _f32, tag="wost")
                nc.vector.tensor_copy(out=wost, in_=ops)
                nc.sync.dma_start(
                    out=out_part[ec * 128:(ec + 1) * 128, st * NT:(st + 1) * NT],
                    in_=wost)

    return nc


# revision 6
# speedup vs baseline: 1.1726x; 1.1726x over previous
"""Multi-head attention (nonstandard softmax normalization) on 8 Trainium2
cores.

Sharding: core c -> (batch c//2, head-group c%2 of 8 heads); each core runs
its 8 heads end-to-end plus the partial Wo product over its 512 feature rows;
the host sums the two partial products per batch.

Per-core design (S=1024, 4 head-pairs, fp16 matmuls):
 - ACT-paced software pipeline: the 64 [128,1024] exps on the scalar engine
   (~64us) and the PE work stream are interleaved so neither starves; PE fill
   work (AV of the previous pair, projections of the next pair, vT chunks)
   slots between QK psum generations.
 - QK runs the two heads of a pair as row-co-executing K=64 matmuls
   (lone K=64 matmuls stream at ~2 cycles/col; the row-disjoint pair
   restores ~1 cycle/col aggregate).
 - denom[t] = sum_u e[t,u] via 4-way column-tiled ones-matmuls (M=32
   replication -> all psum rows valid); per-head accumulation streaks are
   kept disjoint in time (interleaved open groups in one psum bank corrupt).
   A DRAM roundtrip transposes the free-dim denom to partitions; recip is
   folded into vT before the AV matmuls.
 - AV co-executes the two heads as column-tiled M=64 matmuls.
 - tail: while the last pair's denom roundtrip is in flight, 5 Wo tiles
   open partial fc0..2 accumulations in psum banks freed by the QK pools;
   only their fc3 contribution remains after the last AV.
 - psum budget (8 banks): QK gens 2x[128,1024], denom 1, AV 1, shared 2.
"""

import os
import sys
import types

import numpy as np

import concourse.bass as bass
import concourse.mybir as mybir
import concourse.tile as tile
from contextlib import ExitStack
from collections import deque

_f32 = mybir.dt.float32
_f16 = mybir.dt.float16


def _install_ntff_shim():
    """Register the axon NTFF profile hook if the image's antenv lacks it.

    Lets run_bass_kernel_spmd(trace=True) return exec_time_ns. Harmless if
    already present.
    """
    try:
        import antenv.axon_hooks  # noqa: F401
        return
    except ImportError:
        pass
    try:
        import antenv
        from trn_agent_boot.trn_boot import _ntff_profile_via_ctypes
    except ImportError:
        return
    mod = types.ModuleType("antenv.axon_hooks")
    mod._hook = None

    def set_axon_ntff_profile_hook(h):
        mod._hook = h

    def get_axon_ntff_profile_hook():
        return mod._hook

    mod.set_axon_ntff_profile_hook = set_axon_ntff_profile_hook
    mod.get_axon_ntff_profile_hook = get_axon_ntff_profile_hook
    sys.modules["antenv.axon_hooks"] = mod
    antenv.axon_hooks = mod
    for so in ("/opt/axon/libaxon_pjrt.so",):
        if os.path.exists(so):
            try:
                mod._hook = _ntff_profile_via_ctypes(so)
            except Exception:
                mod._hook = None
            break


def _install_drain_patch():
    """Work around this toolchain's walrus rejecting sem waits on Drain.

    TileContext's final drain carries end-of-kernel semaphore waits inline;
    this walrus build encodes Drain as NEURON_ISA_TPB_CTRL_NO_STRUCT and
    fails codegen ("Too many sync wait commands") for ANY inline wait.
    Equivalent semantics: emit the waits as standalone sync-engine wait
    instructions and leave the Drain bare.
    """
    if getattr(tile.TileContext, "_drain_patch_installed", False):
        return
    from concourse.vector_clock import ScopedClock

    def _patched_drain_and_barrier(self, tick_clock, wait_clock):
        drain_inst = self.nc.sync.drain()
        wait_clock.add_sem_waits(
            drain_inst.ins, ScopedClock({None: tick_clock.global_clock})
        )
        si = drain_inst.ins.sync_info
        waits = list(si.on_wait) if si is not None else []
        if waits:
            drain_inst.ins.sync_info = mybir.SyncInfo(
                on_wait=[], on_update=list(si.on_update) if si.on_update else []
            )
            by_name = (
                {h.name: h for h in self.sems.allocated().values()}
                if self.sems is not None else {}
            )
            for w in waits:
                sem = by_name.get(w.ant_name)
                assert sem is not None, f"unknown drain-wait sem: {w.ant_name}"
                assert w.wait_mode == "sem-ge-imm", w
                self.nc.sync.wait_ge(sem, w.wait_value)
        self.nc.all_engine_barrier()
        assert self.sems is not None
        popped = self.nc._tile_sem_poison_stack.pop()
        assert popped is self._sem_poison
        self.nc.clear_and_free_semaphores(list(self.sems.allocated().values()))
        self.nc.all_engine_barrier()

    tile.TileContext._drain_and_barrier = _patched_drain_and_barrier

    # Same walrus limitation, general form: at most ONE inline sem wait per
    # instruction. Tile's wait assignment can attach several (e.g. a DMA
    # waiting on a slot freed by PE + DVE + another queue). Hoist all but
    # the last wait onto same-engine EventSemaphore carrier instructions.
    orig_add = tile.TileContext._add_instruction

    def _split_add_instruction(self, inst):
        si = inst.sync_info
        if si is not None and si.on_wait and len(si.on_wait) > 1:
            waits = list(si.on_wait)
            for w in waits[:-1]:
                ev = mybir.InstEventSemaphore(
                    name=self.nc.get_next_instruction_name(),
                    engine=inst.engine,
                    sync_info=mybir.SyncInfo(on_wait=[w], on_update=[]),
                )
                orig_add(self, ev)
            inst.sync_info = mybir.SyncInfo(
                on_wait=[waits[-1]],
                on_update=list(si.on_update) if si.on_update else [],
            )
        orig_add(self, inst)

    tile.TileContext._add_instruction = _split_add_instruction
    tile.TileContext._drain_patch_installed = True


def build_core_kernel_v2(S=1024, n_pairs=4, e_out=1024, debug=False):
    _install_drain_patch()

    C = S // 128          # 8 t-chunks per pair
    NT = 512              # matmul moving tile (= psum bank)
    NS = S // NT          # 2
    S4 = S // 4           # 256 denom col-group width
    EC = e_out // 128     # 8 output row-chunks
    FP = n_pairs * 128    # 512 feature rows on this core

    nc = bass.Bass()
    q_rows = nc.declare_dram_parameter("q_rows", [FP, S], _f16, isOutput=False)
    k_rows = nc.declare_dram_parameter("k_rows", [FP, S], _f16, isOutput=False)
    v_rows = nc.declare_dram_parameter("v_rows", [FP, S], _f16, isOutput=False)
    # host-side pre-arranged: w{q,k,v}T[p, pr, m] (blockdiag pair weights,
    # partition-major) and woT[p, pr, e] = Wo.T[pr*128+p, e]
    wqT = nc.declare_dram_parameter("wqT", [128, n_pairs, 128], _f16, isOutput=False)
    wkT = nc.declare_dram_parameter("wkT", [128, n_pairs, 128], _f16, isOutput=False)
    wvT = nc.declare_dram_parameter("wvT", [128, n_pairs, 128], _f16, isOutput=False)
    woT = nc.declare_dram_parameter("woT", [128, n_pairs, e_out], _f16, isOutput=False)
    out_part = nc.declare_dram_parameter("out_part", [e_out, S], _f32, isOutput=True)
    if debug:
        dbg_rc = nc.declare_dram_parameter("dbg_rc", [n_pairs, 128, 2, 8], _f32, isOutput=True)
        dbg_E = nc.declare_dram_parameter("dbg_E", [128, 1024], _f16, isOutput=True)
        dbg_vt = nc.declare_dram_parameter("dbg_vt", [n_pairs, 128, 8, 128], _f16, isOutput=True)
        dbg_dstage = nc.declare_dram_parameter("dbg_dstage", [n_pairs, 128, 512], _f32, isOutput=True)

    Exp = mybir.ActivationFunctionType.Exp
    Mult = mybir.AluOpType.mult

    with tile.TileContext(nc) as tc, ExitStack() as ctx:
        consts = ctx.enter_context(tc.tile_pool(name="consts", bufs=1))
        wop = ctx.enter_context(tc.tile_pool(name="wop", bufs=1))
        raws = ctx.enter_context(tc.tile_pool(name="raws", bufs=2))
        qks = ctx.enter_context(tc.tile_pool(name="qks", bufs=2))
        outp = ctx.enter_context(tc.tile_pool(name="outp", bufs=1))
        vts = ctx.enter_context(tc.tile_pool(name="vts", bufs=n_pairs))
        Ep = ctx.enter_context(tc.tile_pool(name="Ep", bufs=4))
        dstp = ctx.enter_context(tc.tile_pool(name="dstp", bufs=2))
        rcp = ctx.enter_context(tc.tile_pool(name="rcp", bufs=2))
        wostp = ctx.enter_context(tc.tile_pool(name="wostp", bufs=3))
        dram = ctx.enter_context(tc.tile_pool(name="dscratch", bufs=2, space="DRAM"))
        # psum: allocation order fixes bank layout; exactly 8 banks
        pqk = ctx.enter_context(tc.tile_pool(name="pqk", bufs=2, space="PSUM"))
        dpsp = ctx.enter_context(tc.tile_pool(name="dpsp", bufs=1, space="PSUM"))
        avpp = ctx.enter_context(tc.tile_pool(name="avpp", bufs=1, space="PSUM"))
        psm = ctx.enter_context(tc.tile_pool(name="psm", bufs=2, space="PSUM"))

        ones32 = consts.tile([128, 32], _f16, tag="ones32")
        nc.vector.memset(ones32, 1.0)
        dummy = consts.tile([128, NT], _f16, tag="dummy")
        nc.vector.memset(dummy, 0.0)
        wq_sb = consts.tile([128, n_pairs, 128], _f16, tag="wq")
        wk_sb = consts.tile([128, n_pairs, 128], _f16, tag="wk")
        wv_sb = consts.tile([128, n_pairs, 128], _f16, tag="wv")

        # ---- input DMAs for pair 0 first, then weights, then the rest ----
        qr_t, kr_t, vr_t = {}, {}, {}

        def issue_raws(pr):
            qr = raws.tile([128, S], _f16, tag="qr", name=f"qr{pr}")
            kr = raws.tile([128, S], _f16, tag="kr", name=f"kr{pr}")
            vr = raws.tile([128, S], _f16, tag="vr", name=f"vr{pr}")
            qr_t[pr], kr_t[pr], vr_t[pr] = qr, kr, vr
            rs = slice(pr * 128, (pr + 1) * 128)
            nc.sync.dma_start(out=qr, in_=q_rows[rs, :])
            nc.sync.dma_start(out=kr, in_=k_rows[rs, :])
            nc.sync.dma_start(out=vr, in_=v_rows[rs, :])

        issue_raws(0)
        nc.sync.dma_start(out=wq_sb, in_=wqT[:, :, :])
        nc.sync.dma_start(out=wk_sb, in_=wkT[:, :, :])
        nc.sync.dma_start(out=wv_sb, in_=wvT[:, :, :])
        issue_raws(1)
        woT_sb = wop.tile([128, n_pairs, e_out], _f16, tag="woT")
        nc.sync.dma_start(out=woT_sb, in_=woT[:, :, :])

        # ---- PE warm-up during the DMA lead-in ----
        for _ in range(4):
            ps = psm.tile([128, NT], _f32, tag="ps")
            nc.tensor.matmul(ps[0:32, :], lhsT=ones32, rhs=dummy,
                             start=True, stop=True)

        q_all_t, k_all_t, vt_t, E_t = {}, {}, {}, {}
        out_all = outp.tile([128, n_pairs, S], _f16, tag="outall")

        # ---------- emission helpers ----------
        def proj_unit(pr, which, st):
            """One [128,512] projection matmul + copy into q_all/k_all."""
            if which == "q":
                src, wt = qr_t[pr], wq_sb
                if pr not in q_all_t:
                    q_all_t[pr] = qks.tile([128, S], _f16, tag="qa", name=f"qa{pr}")
                dst = q_all_t[pr]
            else:
                src, wt = kr_t[pr], wk_sb
                if pr not in k_all_t:
                    k_all_t[pr] = qks.tile([128, S], _f16, tag="ka", name=f"ka{pr}")
                dst = k_all_t[pr]
            ps = psm.tile([128, NT], _f32, tag="ps")
            nc.tensor.matmul(ps, lhsT=wt[:, pr, :],
                             rhs=src[:, st * NT:(st + 1) * NT],
                             start=True, stop=True)
            nc.vector.tensor_copy(out=dst[:, st * NT:(st + 1) * NT], in_=ps)

        def vt_unit(pr, c):
            """vT chunk c: [128t,128i] via transpose-projection matmul."""
            if pr not in vt_t:
                vt_t[pr] = vts.tile([128, C, 128], _f16, tag="vt", name=f"vt{pr}")
            vrc = vr_t[pr].rearrange("p (c t) -> p c t", c=C)
            ps = psm.tile([128, NT], _f32, tag="ps")
            nc.tensor.matmul(ps[:, 0:128], lhsT=vrc[:, c, :],
                             rhs=wv_sb[:, pr, :], start=True, stop=True)
            nc.vector.tensor_copy(out=vt_t[pr][:, c, :], in_=ps[:, 0:128])

        def qk_pair_gen(pr, c):
            """QK chunk c, both heads row-co-executing, + the two exps.

            K=64 matmuls run ~2 cycles/col alone; interleaving the two heads
            (disjoint PE row halves) restores ~1 cycle/col aggregate. Order
            (h0,st0),(h1,st0),(h1,st1),(h0,st1) chains same-stationary pairs.
            """
            ka = k_all_t[pr]
            qa = q_all_t[pr]
            pst = {hh: pqk.tile([128, S], _f32, tag="pqk",
                                name=f"pqk_{pr}_{c}_{hh}") for hh in (0, 1)}

            def mm(hh, st):
                nc.tensor.matmul(
                    pst[hh][:, st * NT:(st + 1) * NT],
                    lhsT=ka[64 * hh:64 * hh + 64, c * 128:(c + 1) * 128],
                    rhs=qa[64 * hh:64 * hh + 64, st * NT:(st + 1) * NT],
                    start=True, stop=True)

            mm(0, 0)
            mm(1, 0)
            mm(1, 1)
            mm(0, 1)
            for hh in (0, 1):
                nc.scalar.activation(out=E_t[(pr, hh)][:, c, :], in_=pst[hh],
                                     func=Exp, scale=0.125)

        dps_t = {}

        def denom_gen(pr, hh, c):
            """4-way col-tiled ones-matmuls (fast co-exec); h0/h1 share the
            bank col-split, so the two heads' groups must NOT interleave in
            time (emit h0's full c-streak, then h1's)."""
            if pr not in dps_t:
                dps_t[pr] = dpsp.tile([128, NT], _f32, tag="dps", name=f"dps{pr}")
            dps = dps_t[pr]
            E = E_t[(pr, hh)]
            for q4 in range(4):
                nc.tensor.matmul(
                    dps[32 * q4:32 * q4 + 32, 256 * hh:256 * hh + 256],
                    lhsT=ones32,
                    rhs=E[:, c, q4 * S4:(q4 + 1) * S4],
                    start=(c == 0), stop=(c == C - 1),
                    tile_position=(0, 32 * q4),
                    skip_group_check=True)

        def roundtrip_scale(pr):
            """denom psum -> DRAM transpose -> recip -> fold into vT."""
            dstage = dstp.tile([128, 2, 256], _f32, tag="dst", name=f"dst{pr}")
            nc.vector.tensor_copy(out=dstage, in_=dps_t[pr].rearrange(
                "p (h x) -> p h x", h=2))
            scr = dram.tile([2048], _f32, tag="scr", name=f"scr{pr}")
            # scr[h*1024 + q4*256 + x] = denom_h[q4*256 + x]
            nc.sync.dma_start(
                out=scr.rearrange("(h a f) -> a h f", h=2, a=4),
                in_=dstage[0:97:32, :, :])
            rcr = rcp.tile([128, 2, C], _f32, tag="rcr", name=f"rcr{pr}")
            rc = rcp.tile([128, 2, C], _f32, tag="rc", name=f"rc{pr}")
            nc.sync.dma_start(
                out=rcr,
                in_=scr.rearrange("(h c p) -> p h c", h=2, p=128))
            nc.vector.reciprocal(out=rc, in_=rcr)
            vt = vt_t[pr]
            for hh in (0, 1):
                nc.vector.tensor_tensor(
                    out=vt[:, :, 64 * hh:64 * hh + 64],
                    in0=vt[:, :, 64 * hh:64 * hh + 64],
                    in1=rc[:, hh, :, None].to_broadcast((128, C, 64)),
                    op=Mult)
            if debug:
                nc.sync.dma_start(out=dbg_rc[pr], in_=rc)
                nc.sync.dma_start(out=dbg_dstage[pr], in_=dstage)
                nc.sync.dma_start(out=dbg_vt[pr], in_=vt)
                if pr == 0:
                    nc.sync.dma_start(out=dbg_E[:, :], in_=E_t[(0, 0)][:, 0, :])

        av_t = {}

        def av_unit(pr, st, c0, nch=2):
            """AV chunks [c0, c0+nch) chained so vt stationary loads hide
            behind the previous co-exec pair's stream."""
            if (pr, st) not in av_t:
                av_t[(pr, st)] = avpp.tile([128, NT], _f32, tag="avp",
                                           name=f"avp{pr}_{st}")
            avp = av_t[(pr, st)]
            for c in range(c0, c0 + nch):
                for hh in (0, 1):
                    nc.tensor.matmul(
                        avp[64 * hh:64 * hh + 64, :],
                        lhsT=vt_t[pr][:, c, 64 * hh:64 * hh + 64],
                        rhs=E_t[(pr, hh)][:, c, st * NT:(st + 1) * NT],
                        start=(c == 0), stop=(c == C - 1),
                        tile_position=(0, 64 * hh),
                        skip_group_check=True)

        def av_copy(pr, st):
            nc.vector.tensor_copy(
                out=out_all[:, pr, st * NT:(st + 1) * NT], in_=av_t[(pr, st)])

        # ---------- the pipeline ----------
        for pr in range(n_pairs):
            E_t[(pr, 0)] = Ep.tile([128, C, S], _f16, tag="E", name=f"E{pr}_0")
            E_t[(pr, 1)] = Ep.tile([128, C, S], _f16, tag="E", name=f"E{pr}_1")

        # initial projections for pair 0 (not hidden behind anything);
        # st0 q+k first so QK chunk 0 can start before st1 lands
        proj_unit(0, "q", 0)
        proj_unit(0, "k", 0)
        proj_unit(0, "q", 1)
        proj_unit(0, "k", 1)

        for pr in range(n_pairs):
            # fill units consumed across this pair's 16 QK gens
            # fill order matters: AV of pair p-1 depends on its denom
            # roundtrip (~3.3us latency from end of pair p-1) -- vt/proj
            # units must come first or the in-order PE queue stalls on AV
            fill = deque()
            for c in range(C):
                fill.append(lambda pr=pr, c=c: vt_unit(pr, c))
            if pr + 1 < n_pairs:
                fill.append(lambda pr=pr: proj_unit(pr + 1, "q", 0))
                fill.append(lambda pr=pr: proj_unit(pr + 1, "k", 0))
                fill.append(lambda pr=pr: proj_unit(pr + 1, "q", 1))
                fill.append(lambda pr=pr: proj_unit(pr + 1, "k", 1))
            if pr >= 1:
                for st in range(NS):
                    for c0 in range(C):
                        fill.append(
                            lambda pr=pr, st=st, c0=c0: av_unit(pr - 1, st, c0,
                                                                nch=1))
                    fill.append(lambda pr=pr, st=st: av_copy(pr - 1, st))
            if pr + 2 < n_pairs:
                fill.append(lambda pr=pr: issue_raws(pr + 2))

            total = len(fill)
            done = 0
            for c in range(C):
                qk_pair_gen(pr, c)
                want = (total * (c + 1)) // C
                while done < want:
                    fill.popleft()()
                    done += 1
            while fill:
                fill.popleft()()
            for hh in (0, 1):
                for c in range(C):
                    denom_gen(pr, hh, c)
            roundtrip_scale(pr)

        # tail: while pair 3's denom roundtrip is in flight (PE would idle),
        # open partial Wo accumulations for 6 (st, ec) tiles over fc 0..2 in
        # psum banks freed by the QK/denom pools; their fc3 lands after AV p3.
        pr = n_pairs - 1

        def wo_mm(ops, fc, ec, st, start, stop):
            for mh in (0, 1):
                nc.tensor.matmul(
                    ops[64 * mh:64 * mh + 64, :],
                    lhsT=woT_sb[:, fc,
                                ec * 128 + 64 * mh:ec * 128 + 64 * mh + 64],
                    rhs=out_all[:, fc, st * NT:(st + 1) * NT],
                    start=start, stop=stop,
                    tile_position=(0, 64 * mh),
                    skip_group_check=True)

        part_tiles = {}
        holders = []
        for i in range(2):
            t = pqk.tile([128, S], _f32, tag="pqk", name=f"wopart{i}")
            holders.append(t)
        holders.append(dpsp.tile([128, NT], _f32, tag="dps", name="wopart2"))
        slots = [holders[0][:, 0:NT], holders[0][:, NT:S],
                 holders[1][:, 0:NT], holders[1][:, NT:S],
                 holders[2]]
        part_list = [(st, ec) for st in range(NS) for ec in range(3)][:5]
        for slot, (st, ec) in zip(slots, part_list):
            part_tiles[(st, ec)] = slot
            for fc in range(3):
                wo_mm(slot, fc, ec, st, start=(fc == 0), stop=False)

        for st in range(NS):
            av_unit(pr, st, 0, nch=C)
            av_copy(pr, st)

        for st in range(NS):
            for ec in range(EC):
                if (st, ec) in part_tiles:
                    ops = part_tiles[(st, ec)]
                    wo_mm(ops, n_pairs - 1, ec, st, start=False, stop=True)
                else:
                    ops = psm.tile([128, NT], _f32, tag="ps")
                    for fc in range(n_pairs):
                        wo_mm(ops, fc, ec, st,
                              start=(fc == 0), stop=(fc == n_pairs - 1))
                wost = wostp.tile([128, NT], _f32, tag="wost")
                nc.vector.tensor_copy(out=wost, in_=ops)
                nc.sync.dma_start(
                    out=out_part[ec * 128:(ec + 1) * 128, st * NT:(st + 1) * NT],
                    in_=wost)

    return nc


# revision 8
# speedup vs baseline: 1.2125x; 1.0340x over previous
"""Multi-head attention (nonstandard softmax normalization) on 8 Trainium2
cores.

Sharding: core c -> (batch c//2, head-group c%2 of 8 heads); each core runs
its 8 heads end-to-end plus the partial Wo product over its 512 feature rows;
the host sums the two partial products per batch.

Per-core design (S=1024, 4 head-pairs, fp16 matmuls):
 - ACT-paced software pipeline: the 64 [128,1024] exps on the scalar engine
   (~64us) and the PE work stream are interleaved so neither starves; PE fill
   work (AV of the previous pair, projections of the next pair, vT chunks)
   slots between QK psum generations.
 - QK runs the two heads of a pair as row-co-executing K=64 matmuls
   (lone K=64 matmuls stream at ~2 cycles/col; the row-disjoint pair
   restores ~1 cycle/col aggregate).
 - denom[t] = sum_u e[t,u] via 4-way column-tiled ones-matmuls (M=32
   replication -> all psum rows valid); per-head accumulation streaks are
   kept disjoint in time (interleaved open groups in one psum bank corrupt).
   A DRAM roundtrip transposes the free-dim denom to partitions; recip is
   folded into vT before the AV matmuls.
 - AV co-executes the two heads as column-tiled M=64 matmuls.
 - tail: while the last pair's denom roundtrip is in flight, 5 Wo tiles
   open partial fc0..2 accumulations in psum banks freed by the QK pools;
   only their fc3 contribution remains after the last AV.
 - psum budget (8 banks): QK gens 2x[128,1024], denom 1, AV 1, shared 2.
"""

import os
import sys
import types

import numpy as np

import concourse.bass as bass
import concourse.mybir as mybir
import concourse.tile as tile
from contextlib import ExitStack
from collections import deque

_f32 = mybir.dt.float32
_f16 = mybir.dt.float16


def _install_ntff_shim():
    """Register the axon NTFF profile hook if the image's antenv lacks it.

    Lets run_bass_kernel_spmd(trace=True) return exec_time_ns. Harmless if
    already present.
    """
    try:
        import antenv.axon_hooks  # noqa: F401
        return
    except ImportError:
        pass
    try:
        import antenv
        from trn_agent_boot.trn_boot import _ntff_profile_via_ctypes
    except ImportError:
        return
    mod = types.ModuleType("antenv.axon_hooks")
    mod._hook = None

    def set_axon_ntff_profile_hook(h):
        mod._hook = h

    def get_axon_ntff_profile_hook():
        return mod._hook

    mod.set_axon_ntff_profile_hook = set_axon_ntff_profile_hook
    mod.get_axon_ntff_profile_hook = get_axon_ntff_profile_hook
    sys.modules["antenv.axon_hooks"] = mod
    antenv.axon_hooks = mod
    for so in ("/opt/axon/libaxon_pjrt.so",):
        if os.path.exists(so):
            try:
                mod._hook = _ntff_profile_via_ctypes(so)
            except Exception:
                mod._hook = None
            break


def _install_drain_patch():
    """Work around this toolchain's walrus rejecting sem waits on Drain.

    TileContext's final drain carries end-of-kernel semaphore waits inline;
    this walrus build encodes Drain as NEURON_ISA_TPB_CTRL_NO_STRUCT and
    fails codegen ("Too many sync wait commands") for ANY inline wait.
    Equivalent semantics: emit the waits as standalone sync-engine wait
    instructions and leave the Drain bare.
    """
    if getattr(tile.TileContext, "_drain_patch_installed", False):
        return
    from concourse.vector_clock import ScopedClock

    def _patched_drain_and_barrier(self, tick_clock, wait_clock):
        drain_inst = self.nc.sync.drain()
        wait_clock.add_sem_waits(
            drain_inst.ins, ScopedClock({None: tick_clock.global_clock})
        )
        si = drain_inst.ins.sync_info
        waits = list(si.on_wait) if si is not None else []
        if waits:
            drain_inst.ins.sync_info = mybir.SyncInfo(
                on_wait=[], on_update=list(si.on_update) if si.on_update else []
            )
            by_name = (
                {h.name: h for h in self.sems.allocated().values()}
                if self.sems is not None else {}
            )
            for w in waits:
                sem = by_name.get(w.ant_name)
                assert sem is not None, f"unknown drain-wait sem: {w.ant_name}"
                assert w.wait_mode == "sem-ge-imm", w
                self.nc.sync.wait_ge(sem, w.wait_value)
        self.nc.all_engine_barrier()
        assert self.sems is not None
        popped = self.nc._tile_sem_poison_stack.pop()
        assert popped is self._sem_poison
        self.nc.clear_and_free_semaphores(list(self.sems.allocated().values()))
        self.nc.all_engine_barrier()

    tile.TileContext._drain_and_barrier = _patched_drain_and_barrier

    # Same walrus limitation, general form: at most ONE inline sem wait per
    # instruction. Tile's wait assignment can attach several (e.g. a DMA
    # waiting on a slot freed by PE + DVE + another queue). Hoist all but
    # the last wait onto same-engine EventSemaphore carrier instructions.
    orig_add = tile.TileContext._add_instruction

    def _split_add_instruction(self, inst):
        si = inst.sync_info
        if si is not None and si.on_wait and len(si.on_wait) > 1:
            waits = list(si.on_wait)
            for w in waits[:-1]:
                ev = mybir.InstEventSemaphore(
                    name=self.nc.get_next_instruction_name(),
                    engine=inst.engine,
                    sync_info=mybir.SyncInfo(on_wait=[w], on_update=[]),
                )
                orig_add(self, ev)
            inst.sync_info = mybir.SyncInfo(
                on_wait=[waits[-1]],
                on_update=list(si.on_update) if si.on_update else [],
            )
        orig_add(self, inst)

    tile.TileContext._add_instruction = _split_add_instruction
    tile.TileContext._drain_patch_installed = True


def build_core_kernel_v2(S=1024, n_pairs=4, e_out=1024, debug=False):
    _install_drain_patch()

    C = S // 128          # 8 t-chunks per pair
    NT = 512              # matmul moving tile (= psum bank)
    NS = S // NT          # 2
    S4 = S // 4           # 256 denom col-group width
    EC = e_out // 128     # 8 output row-chunks
    FP = n_pairs * 128    # 512 feature rows on this core

    nc = bass.Bass()
    q_rows = nc.declare_dram_parameter("q_rows", [FP, S], _f16, isOutput=False)
    k_rows = nc.declare_dram_parameter("k_rows", [FP, S], _f16, isOutput=False)
    v_rows = nc.declare_dram_parameter("v_rows", [FP, S], _f16, isOutput=False)
    # host-side pre-arranged: w{q,k,v}T[p, pr, m] (blockdiag pair weights,
    # partition-major) and woT[p, pr, e] = Wo.T[pr*128+p, e]
    wqT = nc.declare_dram_parameter("wqT", [128, n_pairs, 128], _f16, isOutput=False)
    wkT = nc.declare_dram_parameter("wkT", [128, n_pairs, 128], _f16, isOutput=False)
    wvT = nc.declare_dram_parameter("wvT", [128, n_pairs, 128], _f16, isOutput=False)
    woT = nc.declare_dram_parameter("woT", [128, n_pairs, e_out], _f16, isOutput=False)
    out_part = nc.declare_dram_parameter("out_part", [e_out, S], _f32, isOutput=True)
    if debug:
        dbg_rc = nc.declare_dram_parameter("dbg_rc", [n_pairs, 128, 2, 8], _f32, isOutput=True)
        dbg_E = nc.declare_dram_parameter("dbg_E", [128, 1024], _f16, isOutput=True)
        dbg_vt = nc.declare_dram_parameter("dbg_vt", [n_pairs, 128, 8, 128], _f16, isOutput=True)
        dbg_dstage = nc.declare_dram_parameter("dbg_dstage", [n_pairs, 128, 512], _f32, isOutput=True)

    Exp = mybir.ActivationFunctionType.Exp
    Mult = mybir.AluOpType.mult

    with tile.TileContext(nc) as tc, ExitStack() as ctx:
        consts = ctx.enter_context(tc.tile_pool(name="consts", bufs=1))
        wop = ctx.enter_context(tc.tile_pool(name="wop", bufs=1))
        raws = ctx.enter_context(tc.tile_pool(name="raws", bufs=2))
        qks = ctx.enter_context(tc.tile_pool(name="qks", bufs=2))
        outp = ctx.enter_context(tc.tile_pool(name="outp", bufs=1))
        vts = ctx.enter_context(tc.tile_pool(name="vts", bufs=n_pairs))
        Ep = ctx.enter_context(tc.tile_pool(name="Ep", bufs=4))
        dstp = ctx.enter_context(tc.tile_pool(name="dstp", bufs=2))
        rcp = ctx.enter_context(tc.tile_pool(name="rcp", bufs=2))
        wostp = ctx.enter_context(tc.tile_pool(name="wostp", bufs=3))
        dram = ctx.enter_context(tc.tile_pool(name="dscratch", bufs=2, space="DRAM"))
        # psum: allocation order fixes bank layout; exactly 8 banks
        pqk = ctx.enter_context(tc.tile_pool(name="pqk", bufs=2, space="PSUM"))
        dpsp = ctx.enter_context(tc.tile_pool(name="dpsp", bufs=1, space="PSUM"))
        avpp = ctx.enter_context(tc.tile_pool(name="avpp", bufs=1, space="PSUM"))
        psm = ctx.enter_context(tc.tile_pool(name="psm", bufs=2, space="PSUM"))

        ones32 = consts.tile([128, 32], _f16, tag="ones32")
        nc.vector.memset(ones32, 1.0)
        dummy = consts.tile([128, NT], _f16, tag="dummy")
        nc.vector.memset(dummy, 0.0)
        wq_sb = consts.tile([128, n_pairs, 128], _f16, tag="wq")
        wk_sb = consts.tile([128, n_pairs, 128], _f16, tag="wk")
        wv_sb = consts.tile([128, n_pairs, 128], _f16, tag="wv")

        # ---- input DMAs for pair 0 first, then weights, then the rest ----
        qr_t, kr_t, vr_t = {}, {}, {}

        def issue_raws(pr):
            qr = raws.tile([128, S], _f16, tag="qr", name=f"qr{pr}")
            kr = raws.tile([128, S], _f16, tag="kr", name=f"kr{pr}")
            vr = raws.tile([128, S], _f16, tag="vr", name=f"vr{pr}")
            qr_t[pr], kr_t[pr], vr_t[pr] = qr, kr, vr
            rs = slice(pr * 128, (pr + 1) * 128)
            nc.sync.dma_start(out=qr, in_=q_rows[rs, :])
            nc.sync.dma_start(out=kr, in_=k_rows[rs, :])
            nc.sync.dma_start(out=vr, in_=v_rows[rs, :])

        issue_raws(0)
        nc.sync.dma_start(out=wq_sb, in_=wqT[:, :, :])
        nc.sync.dma_start(out=wk_sb, in_=wkT[:, :, :])
        nc.sync.dma_start(out=wv_sb, in_=wvT[:, :, :])
        issue_raws(1)
        woT_sb = wop.tile([128, n_pairs, e_out], _f16, tag="woT")
        nc.sync.dma_start(out=woT_sb, in_=woT[:, :, :])

        # ---- PE warm-up during the DMA lead-in ----
        for _ in range(4):
            ps = psm.tile([128, NT], _f32, tag="ps")
            nc.tensor.matmul(ps[0:32, :], lhsT=ones32, rhs=dummy,
                             start=True, stop=True)

        q_all_t, k_all_t, vt_t, E_t = {}, {}, {}, {}
        out_all = outp.tile([128, n_pairs, S], _f16, tag="outall")

        # ---------- emission helpers ----------
        def proj_unit(pr, which, st):
            """One [128,512] projection matmul + copy into q_all/k_all."""
            if which == "q":
                src, wt = qr_t[pr], wq_sb
                if pr not in q_all_t:
                    q_all_t[pr] = qks.tile([128, S], _f16, tag="qa", name=f"qa{pr}")
                dst = q_all_t[pr]
            else:
                src, wt = kr_t[pr], wk_sb
                if pr not in k_all_t:
                    k_all_t[pr] = qks.tile([128, S], _f16, tag="ka", name=f"ka{pr}")
                dst = k_all_t[pr]
            ps = psm.tile([128, NT], _f32, tag="ps")
            nc.tensor.matmul(ps, lhsT=wt[:, pr, :],
                             rhs=src[:, st * NT:(st + 1) * NT],
                             start=True, stop=True)
            nc.vector.tensor_copy(out=dst[:, st * NT:(st + 1) * NT], in_=ps)

        def vt_unit(pr, c):
            """vT chunk c: [128t,128i] via transpose-projection matmul."""
            if pr not in vt_t:
                vt_t[pr] = vts.tile([128, C, 128], _f16, tag="vt", name=f"vt{pr}")
            vrc = vr_t[pr].rearrange("p (c t) -> p c t", c=C)
            ps = psm.tile([128, NT], _f32, tag="ps")
            nc.tensor.matmul(ps[:, 0:128], lhsT=vrc[:, c, :],
                             rhs=wv_sb[:, pr, :], start=True, stop=True)
            nc.vector.tensor_copy(out=vt_t[pr][:, c, :], in_=ps[:, 0:128])

        def qk_pair_gen(pr, c):
            """QK chunk c, both heads row-co-executing, + the two exps.

            K=64 matmuls run ~2 cycles/col alone; interleaving the two heads
            (disjoint PE row halves) restores ~1 cycle/col aggregate. Order
            (h0,st0),(h1,st0),(h1,st1),(h0,st1) chains same-stationary pairs.
            """
            ka = k_all_t[pr]
            qa = q_all_t[pr]
            pst = {hh: pqk.tile([128, S], _f32, tag="pqk",
                                name=f"pqk_{pr}_{c}_{hh}") for hh in (0, 1)}

            def mm(hh, st):
                nc.tensor.matmul(
                    pst[hh][:, st * NT:(st + 1) * NT],
                    lhsT=ka[64 * hh:64 * hh + 64, c * 128:(c + 1) * 128],
                    rhs=qa[64 * hh:64 * hh + 64, st * NT:(st + 1) * NT],
                    start=True, stop=True)

            mm(0, 0)
            mm(1, 0)
            mm(1, 1)
            mm(0, 1)
            for hh in (0, 1):
                nc.scalar.activation(out=E_t[(pr, hh)][:, c, :], in_=pst[hh],
                                     func=Exp, scale=0.125)

        dps_t = {}

        def denom_gen(pr, hh, c):
            """4-way col-tiled ones-matmuls (fast co-exec); h0/h1 share the
            bank col-split, so the two heads' groups must NOT interleave in
            time (emit h0's full c-streak, then h1's)."""
            if pr not in dps_t:
                dps_t[pr] = dpsp.tile([128, NT], _f32, tag="dps", name=f"dps{pr}")
            dps = dps_t[pr]
            E = E_t[(pr, hh)]
            for q4 in range(4):
                nc.tensor.matmul(
                    dps[32 * q4:32 * q4 + 32, 256 * hh:256 * hh + 256],
                    lhsT=ones32,
                    rhs=E[:, c, q4 * S4:(q4 + 1) * S4],
                    start=(c == 0), stop=(c == C - 1),
                    tile_position=(0, 32 * q4),
                    skip_group_check=True)

        def roundtrip_scale(pr):
            """denom psum -> DRAM transpose -> recip -> fold into vT."""
            dstage = dstp.tile([128, 2, 256], _f32, tag="dst", name=f"dst{pr}")
            nc.vector.tensor_copy(out=dstage, in_=dps_t[pr].rearrange(
                "p (h x) -> p h x", h=2))
            scr = dram.tile([2048], _f32, tag="scr", name=f"scr{pr}")
            # scr[h*1024 + q4*256 + x] = denom_h[q4*256 + x]
            nc.sync.dma_start(
                out=scr.rearrange("(h a f) -> a h f", h=2, a=4),
                in_=dstage[0:97:32, :, :])
            rcr = rcp.tile([128, 2, C], _f32, tag="rcr", name=f"rcr{pr}")
            rc = rcp.tile([128, 2, C], _f32, tag="rc", name=f"rc{pr}")
            nc.sync.dma_start(
                out=rcr,
                in_=scr.rearrange("(h c p) -> p h c", h=2, p=128))
            nc.vector.reciprocal(out=rc, in_=rcr)
            vt = vt_t[pr]
            for hh in (0, 1):
                nc.vector.tensor_tensor(
                    out=vt[:, :, 64 * hh:64 * hh + 64],
                    in0=vt[:, :, 64 * hh:64 * hh + 64],
                    in1=rc[:, hh, :, None].to_broadcast((128, C, 64)),
                    op=Mult)
            if debug:
                nc.sync.dma_start(out=dbg_rc[pr], in_=rc)
                nc.sync.dma_start(out=dbg_dstage[pr], in_=dstage)
                nc.sync.dma_start(out=dbg_vt[pr], in_=vt)
                if pr == 0:
                    nc.sync.dma_start(out=dbg_E[:, :], in_=E_t[(0, 0)][:, 0, :])

        av_t = {}

        def av_unit(pr, st, c0, nch=2):
            """AV chunks [c0, c0+nch) chained so vt stationary loads hide
            behind the previous co-exec pair's stream."""
            if (pr, st) not in av_t:
                av_t[(pr, st)] = avpp.tile([128, NT], _f32, tag="avp",
                                           name=f"avp{pr}_{st}")
            avp = av_t[(pr, st)]
            for c in range(c0, c0 + nch):
                for hh in (0, 1):
                    nc.tensor.matmul(
                        avp[64 * hh:64 * hh + 64, :],
                        lhsT=vt_t[pr][:, c, 64 * hh:64 * hh + 64],
                        rhs=E_t[(pr, hh)][:, c, st * NT:(st + 1) * NT],
                        start=(c == 0), stop=(c == C - 1),
                        tile_position=(0, 64 * hh),
                        skip_group_check=True)

        def av_copy(pr, st):
            nc.vector.tensor_copy(
                out=out_all[:, pr, st * NT:(st + 1) * NT], in_=av_t[(pr, st)])

        # ---------- the pipeline ----------
        for pr in range(n_pairs):
            E_t[(pr, 0)] = Ep.tile([128, C, S], _f16, tag="E", name=f"E{pr}_0")
            E_t[(pr, 1)] = Ep.tile([128, C, S], _f16, tag="E", name=f"E{pr}_1")

        # initial projections for pair 0 (not hidden behind anything);
        # st0 q+k first so QK chunk 0 can start before st1 lands
        proj_unit(0, "q", 0)
        proj_unit(0, "k", 0)
        proj_unit(0, "q", 1)
        proj_unit(0, "k", 1)

        for pr in range(n_pairs):
            # fill units consumed across this pair's 16 QK gens
            # fill order matters: AV of pair p-1 depends on its denom
            # roundtrip (~3.3us latency from end of pair p-1) -- vt/proj
            # units must come first or the in-order PE queue stalls on AV
            fill = deque()
            if pr > 0:
                # vt units first (vr landed during the previous pair)
                for c in range(C):
                    fill.append(lambda pr=pr, c=c: vt_unit(pr, c))
            if pr + 1 < n_pairs:
                fill.append(lambda pr=pr: proj_unit(pr + 1, "q", 0))
                fill.append(lambda pr=pr: proj_unit(pr + 1, "k", 0))
                fill.append(lambda pr=pr: proj_unit(pr + 1, "q", 1))
                fill.append(lambda pr=pr: proj_unit(pr + 1, "k", 1))
            if pr >= 1:
                for st in range(NS):
                    for c0 in range(C):
                        fill.append(
                            lambda pr=pr, st=st, c0=c0: av_unit(pr - 1, st, c0,
                                                                nch=1))
                    fill.append(lambda pr=pr, st=st: av_copy(pr - 1, st))
            if pr + 2 < n_pairs:
                fill.append(lambda pr=pr: issue_raws(pr + 2))
            if pr == 0:
                # vt last on pair 0: its vr/wv DMAs land after qr/kr, and an
                # early vt unit head-of-line blocks the whole PE queue
                for c in range(C):
                    fill.append(lambda pr=pr, c=c: vt_unit(pr, c))

            total = len(fill)
            done = 0
            for c in range(C):
                qk_pair_gen(pr, c)
                want = (total * (c + 1)) // C
                while done < want:
                    fill.popleft()()
                    done += 1
            while fill:
                fill.popleft()()
            for hh in (0, 1):
                for c in range(C):
                    denom_gen(pr, hh, c)
            roundtrip_scale(pr)

        # tail: while pair 3's denom roundtrip is in flight (PE would idle),
        # open partial Wo accumulations for 6 (st, ec) tiles over fc 0..2 in
        # psum banks freed by the QK/denom pools; their fc3 lands after AV p3.
        pr = n_pairs - 1

        def wo_mm(ops, fc, ec, st, start, stop):
            for mh in (0, 1):
                nc.tensor.matmul(
                    ops[64 * mh:64 * mh + 64, :],
                    lhsT=woT_sb[:, fc,
                                ec * 128 + 64 * mh:ec * 128 + 64 * mh + 64],
                    rhs=out_all[:, fc, st * NT:(st + 1) * NT],
                    start=start, stop=stop,
                    tile_position=(0, 64 * mh),
                    skip_group_check=True)

        part_tiles = {}
        holders = []
        for i in range(2):
            t = pqk.tile([128, S], _f32, tag="pqk", name=f"wopart{i}")
            holders.append(t)
        holders.append(dpsp.tile([128, NT], _f32, tag="dps", name="wopart2"))
        slots = [holders[0][:, 0:NT], holders[0][:, NT:S],
                 holders[1][:, 0:NT], holders[1][:, NT:S],
                 holders[2]]
        part_list = [(st, ec) for st in range(NS) for ec in range(3)][:5]
        for slot, (st, ec) in zip(slots, part_list):
            part_tiles[(st, ec)] = slot
            for fc in range(3):
                wo_mm(slot, fc, ec, st, start=(fc == 0), stop=False)

        for st in range(NS):
            av_unit(pr, st, 0, nch=C)
            av_copy(pr, st)

        for st in range(NS):
            for ec in range(EC):
                if (st, ec) in part_tiles:
                    ops = part_tiles[(st, ec)]
                    wo_mm(ops, n_pairs - 1, ec, st, start=False, stop=True)
                else:
                    ops = psm.tile([128, NT], _f32, tag="ps")
                    for fc in range(n_pairs):
                        wo_mm(ops, fc, ec, st,
                              start=(fc == 0), stop=(fc == n_pairs - 1))
                wost = wostp.tile([128, NT], _f32, tag="wost")
                nc.vector.tensor_copy(out=wost, in_=ops)
                nc.sync.dma_start(
                    out=out_part[ec * 128:(ec + 1) * 128, st * NT:(st + 1) * NT],
                    in_=wost)

    return nc


# revision 9
# speedup vs baseline: 1.2472x; 1.0286x over previous
"""Multi-head attention (nonstandard softmax normalization) on 8 Trainium2
cores.

Sharding: core c -> (batch c//2, head-group c%2 of 8 heads); each core runs
its 8 heads end-to-end plus the partial Wo product over its 512 feature rows;
the host sums the two partial products per batch.

Per-core design (S=1024, 4 head-pairs, fp16 matmuls):
 - ACT-paced software pipeline: the 64 [128,1024] exps on the scalar engine
   (~64us) and the PE work stream are interleaved so neither starves; PE fill
   work (AV of the previous pair, projections of the next pair, vT chunks)
   slots between QK psum generations.
 - QK runs the two heads of a pair as row-co-executing K=64 matmuls
   (lone K=64 matmuls stream at ~2 cycles/col; the row-disjoint pair
   restores ~1 cycle/col aggregate).
 - denom[t] = sum_u e[t,u] via 4-way column-tiled ones-matmuls (M=32
   replication -> all psum rows valid); per-head accumulation streaks are
   kept disjoint in time (interleaved open groups in one psum bank corrupt).
   A DRAM roundtrip transposes the free-dim denom to partitions; recip is
   folded into vT before the AV matmuls.
 - AV co-executes the two heads as column-tiled M=64 matmuls.
 - tail: while the last pair's denom roundtrip is in flight, 5 Wo tiles
   open partial fc0..2 accumulations in psum banks freed by the QK pools;
   only their fc3 contribution remains after the last AV.
 - psum budget (8 banks): QK gens 2x[128,1024], denom 1, AV 1, shared 2.
"""

import os
import sys
import types

import numpy as np

import concourse.bass as bass
import concourse.mybir as mybir
import concourse.tile as tile
from contextlib import ExitStack
from collections import deque

_f32 = mybir.dt.float32
_f16 = mybir.dt.float16


def _install_ntff_shim():
    """Register the axon NTFF profile hook if the image's antenv lacks it.

    Lets run_bass_kernel_spmd(trace=True) return exec_time_ns. Harmless if
    already present.
    """
    try:
        import antenv.axon_hooks  # noqa: F401
        return
    except ImportError:
        pass
    try:
        import antenv
        from trn_agent_boot.trn_boot import _ntff_profile_via_ctypes
    except ImportError:
        return
    mod = types.ModuleType("antenv.axon_hooks")
    mod._hook = None

    def set_axon_ntff_profile_hook(h):
        mod._hook = h

    def get_axon_ntff_profile_hook():
        return mod._hook

    mod.set_axon_ntff_profile_hook = set_axon_ntff_profile_hook
    mod.get_axon_ntff_profile_hook = get_axon_ntff_profile_hook
    sys.modules["antenv.axon_hooks"] = mod
    antenv.axon_hooks = mod
    for so in ("/opt/axon/libaxon_pjrt.so",):
        if os.path.exists(so):
            try:
                mod._hook = _ntff_profile_via_ctypes(so)
            except Exception:
                mod._hook = None
            break


def _install_drain_patch():
    """Work around this toolchain's walrus rejecting sem waits on Drain.

    TileContext's final drain carries end-of-kernel semaphore waits inline;
    this walrus build encodes Drain as NEURON_ISA_TPB_CTRL_NO_STRUCT and
    fails codegen ("Too many sync wait commands") for ANY inline wait.
    Equivalent semantics: emit the waits as standalone sync-engine wait
    instructions and leave the Drain bare.
    """
    if getattr(tile.TileContext, "_drain_patch_installed", False):
        return
    from concourse.vector_clock import ScopedClock

    def _patched_drain_and_barrier(self, tick_clock, wait_clock):
        drain_inst = self.nc.sync.drain()
        wait_clock.add_sem_waits(
            drain_inst.ins, ScopedClock({None: tick_clock.global_clock})
        )
        si = drain_inst.ins.sync_info
        waits = list(si.on_wait) if si is not None else []
        if waits:
            drain_inst.ins.sync_info = mybir.SyncInfo(
                on_wait=[], on_update=list(si.on_update) if si.on_update else []
            )
            by_name = (
                {h.name: h for h in self.sems.allocated().values()}
                if self.sems is not None else {}
            )
            for w in waits:
                sem = by_name.get(w.ant_name)
                assert sem is not None, f"unknown drain-wait sem: {w.ant_name}"
                assert w.wait_mode == "sem-ge-imm", w
                self.nc.sync.wait_ge(sem, w.wait_value)
        self.nc.all_engine_barrier()
        assert self.sems is not None
        popped = self.nc._tile_sem_poison_stack.pop()
        assert popped is self._sem_poison
        self.nc.clear_and_free_semaphores(list(self.sems.allocated().values()))
        self.nc.all_engine_barrier()

    tile.TileContext._drain_and_barrier = _patched_drain_and_barrier

    # Same walrus limitation, general form: at most ONE inline sem wait per
    # instruction. Tile's wait assignment can attach several (e.g. a DMA
    # waiting on a slot freed by PE + DVE + another queue). Hoist all but
    # the last wait onto same-engine EventSemaphore carrier instructions.
    orig_add = tile.TileContext._add_instruction

    def _split_add_instruction(self, inst):
        si = inst.sync_info
        if si is not None and si.on_wait and len(si.on_wait) > 1:
            waits = list(si.on_wait)
            for w in waits[:-1]:
                ev = mybir.InstEventSemaphore(
                    name=self.nc.get_next_instruction_name(),
                    engine=inst.engine,
                    sync_info=mybir.SyncInfo(on_wait=[w], on_update=[]),
                )
                orig_add(self, ev)
            inst.sync_info = mybir.SyncInfo(
                on_wait=[waits[-1]],
                on_update=list(si.on_update) if si.on_update else [],
            )
        orig_add(self, inst)

    tile.TileContext._add_instruction = _split_add_instruction
    tile.TileContext._drain_patch_installed = True


def build_core_kernel_v2(S=1024, n_pairs=4, e_out=1024, debug=False):
    _install_drain_patch()

    C = S // 128          # 8 t-chunks per pair
    NT = 512              # matmul moving tile (= psum bank)
    NS = S // NT          # 2
    S4 = S // 4           # 256 denom col-group width
    EC = e_out // 128     # 8 output row-chunks
    FP = n_pairs * 128    # 512 feature rows on this core

    nc = bass.Bass()
    q_rows = nc.declare_dram_parameter("q_rows", [FP, S], _f16, isOutput=False)
    k_rows = nc.declare_dram_parameter("k_rows", [FP, S], _f16, isOutput=False)
    v_rows = nc.declare_dram_parameter("v_rows", [FP, S], _f16, isOutput=False)
    # host-side pre-arranged: w{q,k,v}T[p, pr, m] (blockdiag pair weights,
    # partition-major) and woT[p, pr, e] = Wo.T[pr*128+p, e]
    wqT = nc.declare_dram_parameter("wqT", [128, n_pairs, 128], _f16, isOutput=False)
    wkT = nc.declare_dram_parameter("wkT", [128, n_pairs, 128], _f16, isOutput=False)
    wvT = nc.declare_dram_parameter("wvT", [128, n_pairs, 128], _f16, isOutput=False)
    woT = nc.declare_dram_parameter("woT", [128, n_pairs, e_out], _f16, isOutput=False)
    out_part = nc.declare_dram_parameter("out_part", [e_out, S], _f32, isOutput=True)
    if debug:
        dbg_rc = nc.declare_dram_parameter("dbg_rc", [n_pairs, 128, 2, 8], _f32, isOutput=True)
        dbg_E = nc.declare_dram_parameter("dbg_E", [128, 1024], _f16, isOutput=True)
        dbg_vt = nc.declare_dram_parameter("dbg_vt", [n_pairs, 128, 8, 128], _f16, isOutput=True)
        dbg_dstage = nc.declare_dram_parameter("dbg_dstage", [n_pairs, 128, 512], _f32, isOutput=True)

    Exp = mybir.ActivationFunctionType.Exp
    Mult = mybir.AluOpType.mult

    with tile.TileContext(nc) as tc, ExitStack() as ctx:
        consts = ctx.enter_context(tc.tile_pool(name="consts", bufs=1))
        wop = ctx.enter_context(tc.tile_pool(name="wop", bufs=1))
        raws = ctx.enter_context(tc.tile_pool(name="raws", bufs=2))
        qks = ctx.enter_context(tc.tile_pool(name="qks", bufs=2))
        outp = ctx.enter_context(tc.tile_pool(name="outp", bufs=1))
        vts = ctx.enter_context(tc.tile_pool(name="vts", bufs=n_pairs))
        Ep = ctx.enter_context(tc.tile_pool(name="Ep", bufs=6))
        dstp = ctx.enter_context(tc.tile_pool(name="dstp", bufs=2))
        rcp = ctx.enter_context(tc.tile_pool(name="rcp", bufs=2))
        wostp = ctx.enter_context(tc.tile_pool(name="wostp", bufs=3))
        dram = ctx.enter_context(tc.tile_pool(name="dscratch", bufs=2, space="DRAM"))
        # psum: allocation order fixes bank layout; exactly 8 banks
        pqk = ctx.enter_context(tc.tile_pool(name="pqk", bufs=2, space="PSUM"))
        dpsp = ctx.enter_context(tc.tile_pool(name="dpsp", bufs=1, space="PSUM"))
        avpp = ctx.enter_context(tc.tile_pool(name="avpp", bufs=1, space="PSUM"))
        psm = ctx.enter_context(tc.tile_pool(name="psm", bufs=2, space="PSUM"))

        ones32 = consts.tile([128, 32], _f16, tag="ones32")
        nc.vector.memset(ones32, 1.0)
        dummy = consts.tile([128, NT], _f16, tag="dummy")
        nc.vector.memset(dummy, 0.0)
        wq_sb = consts.tile([128, n_pairs, 128], _f16, tag="wq")
        wk_sb = consts.tile([128, n_pairs, 128], _f16, tag="wk")
        wv_sb = consts.tile([128, n_pairs, 128], _f16, tag="wv")

        # ---- input DMAs for pair 0 first, then weights, then the rest ----
        qr_t, kr_t, vr_t = {}, {}, {}

        def issue_raws(pr):
            qr = raws.tile([128, S], _f16, tag="qr", name=f"qr{pr}")
            kr = raws.tile([128, S], _f16, tag="kr", name=f"kr{pr}")
            vr = raws.tile([128, S], _f16, tag="vr", name=f"vr{pr}")
            qr_t[pr], kr_t[pr], vr_t[pr] = qr, kr, vr
            rs = slice(pr * 128, (pr + 1) * 128)
            nc.sync.dma_start(out=qr, in_=q_rows[rs, :])
            nc.sync.dma_start(out=kr, in_=k_rows[rs, :])
            nc.sync.dma_start(out=vr, in_=v_rows[rs, :])

        issue_raws(0)
        nc.sync.dma_start(out=wq_sb, in_=wqT[:, :, :])
        nc.sync.dma_start(out=wk_sb, in_=wkT[:, :, :])
        nc.sync.dma_start(out=wv_sb, in_=wvT[:, :, :])
        issue_raws(1)
        woT_sb = wop.tile([128, n_pairs, e_out], _f16, tag="woT")
        nc.sync.dma_start(out=woT_sb, in_=woT[:, :, :])

        # ---- PE warm-up during the DMA lead-in ----
        for _ in range(4):
            ps = psm.tile([128, NT], _f32, tag="ps")
            nc.tensor.matmul(ps[0:32, :], lhsT=ones32, rhs=dummy,
                             start=True, stop=True)

        q_all_t, k_all_t, vt_t, E_t = {}, {}, {}, {}
        out_all = outp.tile([128, n_pairs, S], _f16, tag="outall")

        # ---------- emission helpers ----------
        def proj_unit(pr, which, st):
            """One [128,512] projection matmul + copy into q_all/k_all."""
            if which == "q":
                src, wt = qr_t[pr], wq_sb
                if pr not in q_all_t:
                    q_all_t[pr] = qks.tile([128, S], _f16, tag="qa", name=f"qa{pr}")
                dst = q_all_t[pr]
            else:
                src, wt = kr_t[pr], wk_sb
                if pr not in k_all_t:
                    k_all_t[pr] = qks.tile([128, S], _f16, tag="ka", name=f"ka{pr}")
                dst = k_all_t[pr]
            ps = psm.tile([128, NT], _f32, tag="ps")
            nc.tensor.matmul(ps, lhsT=wt[:, pr, :],
                             rhs=src[:, st * NT:(st + 1) * NT],
                             start=True, stop=True)
            nc.vector.tensor_copy(out=dst[:, st * NT:(st + 1) * NT], in_=ps)

        def vt_unit(pr, c):
            """vT chunk c: [128t,128i] via transpose-projection matmul."""
            if pr not in vt_t:
                vt_t[pr] = vts.tile([128, C, 128], _f16, tag="vt", name=f"vt{pr}")
            vrc = vr_t[pr].rearrange("p (c t) -> p c t", c=C)
            ps = psm.tile([128, NT], _f32, tag="ps")
            nc.tensor.matmul(ps[:, 0:128], lhsT=vrc[:, c, :],
                             rhs=wv_sb[:, pr, :], start=True, stop=True)
            nc.vector.tensor_copy(out=vt_t[pr][:, c, :], in_=ps[:, 0:128])

        def qk_pair_gen(pr, c):
            """QK chunk c, both heads row-co-executing, + the two exps.

            K=64 matmuls run ~2 cycles/col alone; interleaving the two heads
            (disjoint PE row halves) restores ~1 cycle/col aggregate. Order
            (h0,st0),(h1,st0),(h1,st1),(h0,st1) chains same-stationary pairs.
            """
            ka = k_all_t[pr]
            qa = q_all_t[pr]
            pst = {hh: pqk.tile([128, S], _f32, tag="pqk",
                                name=f"pqk_{pr}_{c}_{hh}") for hh in (0, 1)}

            def mm(hh, st):
                nc.tensor.matmul(
                    pst[hh][:, st * NT:(st + 1) * NT],
                    lhsT=ka[64 * hh:64 * hh + 64, c * 128:(c + 1) * 128],
                    rhs=qa[64 * hh:64 * hh + 64, st * NT:(st + 1) * NT],
                    start=True, stop=True)

            mm(0, 0)
            mm(1, 0)
            mm(1, 1)
            mm(0, 1)
            for hh in (0, 1):
                nc.scalar.activation(out=E_t[(pr, hh)][:, c, :], in_=pst[hh],
                                     func=Exp, scale=0.125)

        dps_t = {}

        def denom_gen(pr, hh, c):
            """4-way col-tiled ones-matmuls (fast co-exec); h0/h1 share the
            bank col-split, so the two heads' groups must NOT interleave in
            time (emit h0's full c-streak, then h1's)."""
            if pr not in dps_t:
                dps_t[pr] = dpsp.tile([128, NT], _f32, tag="dps", name=f"dps{pr}")
            dps = dps_t[pr]
            E = E_t[(pr, hh)]
            for q4 in range(4):
                nc.tensor.matmul(
                    dps[32 * q4:32 * q4 + 32, 256 * hh:256 * hh + 256],
                    lhsT=ones32,
                    rhs=E[:, c, q4 * S4:(q4 + 1) * S4],
                    start=(c == 0), stop=(c == C - 1),
                    tile_position=(0, 32 * q4),
                    skip_group_check=True)

        def roundtrip_scale(pr):
            """denom psum -> DRAM transpose -> recip -> fold into vT."""
            dstage = dstp.tile([128, 2, 256], _f32, tag="dst", name=f"dst{pr}")
            nc.vector.tensor_copy(out=dstage, in_=dps_t[pr].rearrange(
                "p (h x) -> p h x", h=2))
            scr = dram.tile([2048], _f32, tag="scr", name=f"scr{pr}")
            # scr[h*1024 + q4*256 + x] = denom_h[q4*256 + x]
            nc.sync.dma_start(
                out=scr.rearrange("(h a f) -> a h f", h=2, a=4),
                in_=dstage[0:97:32, :, :])
            rcr = rcp.tile([128, 2, C], _f32, tag="rcr", name=f"rcr{pr}")
            rc = rcp.tile([128, 2, C], _f32, tag="rc", name=f"rc{pr}")
            nc.sync.dma_start(
                out=rcr,
                in_=scr.rearrange("(h c p) -> p h c", h=2, p=128))
            nc.vector.reciprocal(out=rc, in_=rcr)
            vt = vt_t[pr]
            for hh in (0, 1):
                nc.vector.tensor_tensor(
                    out=vt[:, :, 64 * hh:64 * hh + 64],
                    in0=vt[:, :, 64 * hh:64 * hh + 64],
                    in1=rc[:, hh, :, None].to_broadcast((128, C, 64)),
                    op=Mult)
            if debug:
                nc.sync.dma_start(out=dbg_rc[pr], in_=rc)
                nc.sync.dma_start(out=dbg_dstage[pr], in_=dstage)
                nc.sync.dma_start(out=dbg_vt[pr], in_=vt)
                if pr == 0:
                    nc.sync.dma_start(out=dbg_E[:, :], in_=E_t[(0, 0)][:, 0, :])

        av_t = {}

        def av_unit(pr, st, c0, nch=2):
            """AV chunks [c0, c0+nch) chained so vt stationary loads hide
            behind the previous co-exec pair's stream."""
            if (pr, st) not in av_t:
                av_t[(pr, st)] = avpp.tile([128, NT], _f32, tag="avp",
                                           name=f"avp{pr}_{st}")
            avp = av_t[(pr, st)]
            for c in range(c0, c0 + nch):
                for hh in (0, 1):
                    nc.tensor.matmul(
                        avp[64 * hh:64 * hh + 64, :],
                        lhsT=vt_t[pr][:, c, 64 * hh:64 * hh + 64],
                        rhs=E_t[(pr, hh)][:, c, st * NT:(st + 1) * NT],
                        start=(c == 0), stop=(c == C - 1),
                        tile_position=(0, 64 * hh),
                        skip_group_check=True)

        def av_copy(pr, st):
            nc.vector.tensor_copy(
                out=out_all[:, pr, st * NT:(st + 1) * NT], in_=av_t[(pr, st)])

        # ---------- the pipeline ----------
        for pr in range(n_pairs):
            E_t[(pr, 0)] = Ep.tile([128, C, S], _f16, tag="E", name=f"E{pr}_0")
            E_t[(pr, 1)] = Ep.tile([128, C, S], _f16, tag="E", name=f"E{pr}_1")

        # initial projections for pair 0 (not hidden behind anything);
        # st0 q+k first so QK chunk 0 can start before st1 lands
        proj_unit(0, "q", 0)
        proj_unit(0, "k", 0)
        proj_unit(0, "q", 1)
        proj_unit(0, "k", 1)

        for pr in range(n_pairs):
            # fill units consumed across this pair's 16 QK gens
            # fill order matters: AV of pair p-1 depends on its denom
            # roundtrip (~3.3us latency from end of pair p-1) -- vt/proj
            # units must come first or the in-order PE queue stalls on AV
            fill = deque()
            if pr > 0:
                # vt units first (vr landed during the previous pair)
                for c in range(C):
                    fill.append(lambda pr=pr, c=c: vt_unit(pr, c))
            if pr + 1 < n_pairs:
                fill.append(lambda pr=pr: proj_unit(pr + 1, "q", 0))
                fill.append(lambda pr=pr: proj_unit(pr + 1, "k", 0))
                fill.append(lambda pr=pr: proj_unit(pr + 1, "q", 1))
                fill.append(lambda pr=pr: proj_unit(pr + 1, "k", 1))
            if pr >= 1:
                for st in range(NS):
                    for c0 in range(C):
                        fill.append(
                            lambda pr=pr, st=st, c0=c0: av_unit(pr - 1, st, c0,
                                                                nch=1))
                    fill.append(lambda pr=pr, st=st: av_copy(pr - 1, st))
            if pr + 2 < n_pairs:
                fill.append(lambda pr=pr: issue_raws(pr + 2))
            if pr == 0:
                # vt last on pair 0: its vr/wv DMAs land after qr/kr, and an
                # early vt unit head-of-line blocks the whole PE queue
                for c in range(C):
                    fill.append(lambda pr=pr, c=c: vt_unit(pr, c))

            total = len(fill)
            done = 0
            for c in range(C):
                qk_pair_gen(pr, c)
                want = (total * (c + 1)) // C
                while done < want:
                    fill.popleft()()
                    done += 1
            while fill:
                fill.popleft()()
            for hh in (0, 1):
                for c in range(C):
                    denom_gen(pr, hh, c)
            roundtrip_scale(pr)

        # tail: while pair 3's denom roundtrip is in flight (PE would idle),
        # open partial Wo accumulations for 6 (st, ec) tiles over fc 0..2 in
        # psum banks freed by the QK/denom pools; their fc3 lands after AV p3.
        pr = n_pairs - 1

        def wo_mm(ops, fc, ec, st, start, stop):
            for mh in (0, 1):
                nc.tensor.matmul(
                    ops[64 * mh:64 * mh + 64, :],
                    lhsT=woT_sb[:, fc,
                                ec * 128 + 64 * mh:ec * 128 + 64 * mh + 64],
                    rhs=out_all[:, fc, st * NT:(st + 1) * NT],
                    start=start, stop=stop,
                    tile_position=(0, 64 * mh),
                    skip_group_check=True)

        part_tiles = {}
        holders = []
        for i in range(2):
            t = pqk.tile([128, S], _f32, tag="pqk", name=f"wopart{i}")
            holders.append(t)
        holders.append(dpsp.tile([128, NT], _f32, tag="dps", name="wopart2"))
        slots = [holders[0][:, 0:NT], holders[0][:, NT:S],
                 holders[1][:, 0:NT], holders[1][:, NT:S],
                 holders[2]]
        part_list = [(st, ec) for st in range(NS) for ec in range(3)][:5]
        for slot, (st, ec) in zip(slots, part_list):
            part_tiles[(st, ec)] = slot
            for fc in range(3):
                wo_mm(slot, fc, ec, st, start=(fc == 0), stop=False)

        for st in range(NS):
            av_unit(pr, st, 0, nch=C)
            av_copy(pr, st)

        for st in range(NS):
            for ec in range(EC):
                if (st, ec) in part_tiles:
                    ops = part_tiles[(st, ec)]
                    wo_mm(ops, n_pairs - 1, ec, st, start=False, stop=True)
                else:
                    ops = psm.tile([128, NT], _f32, tag="ps")
                    for fc in range(n_pairs):
                        wo_mm(ops, fc, ec, st,
                              start=(fc == 0), stop=(fc == n_pairs - 1))
                wost = wostp.tile([128, NT], _f32, tag="wost")
                nc.vector.tensor_copy(out=wost, in_=ops)
                nc.sync.dma_start(
                    out=out_part[ec * 128:(ec + 1) * 128, st * NT:(st + 1) * NT],
                    in_=wost)

    return nc


# revision 10
# speedup vs baseline: 1.2677x; 1.0164x over previous
"""Multi-head attention (nonstandard softmax normalization) on 8 Trainium2
cores.

Sharding: core c -> (batch c//2, head-group c%2 of 8 heads); each core runs
its 8 heads end-to-end plus the partial Wo product over its 512 feature rows;
the host sums the two partial products per batch.

Per-core design (S=1024, 4 head-pairs, fp16 matmuls):
 - ACT-paced software pipeline: the 64 [128,1024] exps on the scalar engine
   (~64us) and the PE work stream are interleaved so neither starves; PE fill
   work (AV of the previous pair, projections of the next pair, vT chunks)
   slots between QK psum generations.
 - QK runs the two heads of a pair as row-co-executing K=64 matmuls
   (lone K=64 matmuls stream at ~2 cycles/col; the row-disjoint pair
   restores ~1 cycle/col aggregate).
 - denom[t] = sum_u e[t,u] via 4-way column-tiled ones-matmuls (M=32
   replication -> all psum rows valid); per-head accumulation streaks are
   kept disjoint in time (interleaved open groups in one psum bank corrupt).
   A DRAM roundtrip transposes the free-dim denom to partitions; recip is
   folded into vT before the AV matmuls.
 - AV co-executes the two heads as column-tiled M=64 matmuls.
 - tail: while the last pair's denom roundtrip is in flight, 5 Wo tiles
   open partial fc0..2 accumulations in psum banks freed by the QK pools;
   only their fc3 contribution remains after the last AV.
 - psum budget (8 banks): QK gens 2x[128,1024], denom 1, AV 1, shared 2.
"""

import os
import sys
import types

import numpy as np

import concourse.bass as bass
import concourse.mybir as mybir
import concourse.tile as tile
from contextlib import ExitStack
from collections import deque

_f32 = mybir.dt.float32
_f16 = mybir.dt.float16


def _install_ntff_shim():
    """Register the axon NTFF profile hook if the image's antenv lacks it.

    Lets run_bass_kernel_spmd(trace=True) return exec_time_ns. Harmless if
    already present.
    """
    try:
        import antenv.axon_hooks  # noqa: F401
        return
    except ImportError:
        pass
    try:
        import antenv
        from trn_agent_boot.trn_boot import _ntff_profile_via_ctypes
    except ImportError:
        return
    mod = types.ModuleType("antenv.axon_hooks")
    mod._hook = None

    def set_axon_ntff_profile_hook(h):
        mod._hook = h

    def get_axon_ntff_profile_hook():
        return mod._hook

    mod.set_axon_ntff_profile_hook = set_axon_ntff_profile_hook
    mod.get_axon_ntff_profile_hook = get_axon_ntff_profile_hook
    sys.modules["antenv.axon_hooks"] = mod
    antenv.axon_hooks = mod
    for so in ("/opt/axon/libaxon_pjrt.so",):
        if os.path.exists(so):
            try:
                mod._hook = _ntff_profile_via_ctypes(so)
            except Exception:
                mod._hook = None
            break


def _install_drain_patch():
    """Work around this toolchain's walrus rejecting sem waits on Drain.

    TileContext's final drain carries end-of-kernel semaphore waits inline;
    this walrus build encodes Drain as NEURON_ISA_TPB_CTRL_NO_STRUCT and
    fails codegen ("Too many sync wait commands") for ANY inline wait.
    Equivalent semantics: emit the waits as standalone sync-engine wait
    instructions and leave the Drain bare.
    """
    if getattr(tile.TileContext, "_drain_patch_installed", False):
        return
    from concourse.vector_clock import ScopedClock

    def _patched_drain_and_barrier(self, tick_clock, wait_clock):
        drain_inst = self.nc.sync.drain()
        wait_clock.add_sem_waits(
            drain_inst.ins, ScopedClock({None: tick_clock.global_clock})
        )
        si = drain_inst.ins.sync_info
        waits = list(si.on_wait) if si is not None else []
        if waits:
            drain_inst.ins.sync_info = mybir.SyncInfo(
                on_wait=[], on_update=list(si.on_update) if si.on_update else []
            )
            by_name = (
                {h.name: h for h in self.sems.allocated().values()}
                if self.sems is not None else {}
            )
            for w in waits:
                sem = by_name.get(w.ant_name)
                assert sem is not None, f"unknown drain-wait sem: {w.ant_name}"
                assert w.wait_mode == "sem-ge-imm", w
                self.nc.sync.wait_ge(sem, w.wait_value)
        self.nc.all_engine_barrier()
        assert self.sems is not None
        popped = self.nc._tile_sem_poison_stack.pop()
        assert popped is self._sem_poison
        self.nc.clear_and_free_semaphores(list(self.sems.allocated().values()))
        self.nc.all_engine_barrier()

    tile.TileContext._drain_and_barrier = _patched_drain_and_barrier

    # Same walrus limitation, general form: at most ONE inline sem wait per
    # instruction. Tile's wait assignment can attach several (e.g. a DMA
    # waiting on a slot freed by PE + DVE + another queue). Hoist all but
    # the last wait onto same-engine EventSemaphore carrier instructions.
    orig_add = tile.TileContext._add_instruction

    def _split_add_instruction(self, inst):
        si = inst.sync_info
        if si is not None and si.on_wait and len(si.on_wait) > 1:
            waits = list(si.on_wait)
            for w in waits[:-1]:
                ev = mybir.InstEventSemaphore(
                    name=self.nc.get_next_instruction_name(),
                    engine=inst.engine,
                    sync_info=mybir.SyncInfo(on_wait=[w], on_update=[]),
                )
                orig_add(self, ev)
            inst.sync_info = mybir.SyncInfo(
                on_wait=[waits[-1]],
                on_update=list(si.on_update) if si.on_update else [],
            )
        orig_add(self, inst)

    tile.TileContext._add_instruction = _split_add_instruction
    tile.TileContext._drain_patch_installed = True


def build_core_kernel_v2(S=1024, n_pairs=4, e_out=1024, debug=False):
    _install_drain_patch()

    C = S // 128          # 8 t-chunks per pair
    NT = 512              # matmul moving tile (= psum bank)
    NS = S // NT          # 2
    S4 = S // 4           # 256 denom col-group width
    EC = e_out // 128     # 8 output row-chunks
    FP = n_pairs * 128    # 512 feature rows on this core

    nc = bass.Bass()
    q_rows = nc.declare_dram_parameter("q_rows", [FP, S], _f16, isOutput=False)
    k_rows = nc.declare_dram_parameter("k_rows", [FP, S], _f16, isOutput=False)
    v_rows = nc.declare_dram_parameter("v_rows", [FP, S], _f16, isOutput=False)
    # host-side pre-arranged: w{q,k,v}T[p, pr, m] (blockdiag pair weights,
    # partition-major) and woT[p, pr, e] = Wo.T[pr*128+p, e]
    wqT = nc.declare_dram_parameter("wqT", [128, n_pairs, 128], _f16, isOutput=False)
    wkT = nc.declare_dram_parameter("wkT", [128, n_pairs, 128], _f16, isOutput=False)
    wvT = nc.declare_dram_parameter("wvT", [128, n_pairs, 128], _f16, isOutput=False)
    woT = nc.declare_dram_parameter("woT", [128, n_pairs, e_out], _f16, isOutput=False)
    out_part = nc.declare_dram_parameter("out_part", [e_out, S], _f32, isOutput=True)
    if debug:
        dbg_rc = nc.declare_dram_parameter("dbg_rc", [n_pairs, 128, 2, 8], _f32, isOutput=True)
        dbg_E = nc.declare_dram_parameter("dbg_E", [128, 1024], _f16, isOutput=True)
        dbg_vt = nc.declare_dram_parameter("dbg_vt", [n_pairs, 128, 8, 128], _f16, isOutput=True)
        dbg_dstage = nc.declare_dram_parameter("dbg_dstage", [n_pairs, 128, 512], _f32, isOutput=True)

    Exp = mybir.ActivationFunctionType.Exp
    Mult = mybir.AluOpType.mult

    with tile.TileContext(nc) as tc, ExitStack() as ctx:
        consts = ctx.enter_context(tc.tile_pool(name="consts", bufs=1))
        wop = ctx.enter_context(tc.tile_pool(name="wop", bufs=1))
        raws = ctx.enter_context(tc.tile_pool(name="raws", bufs=2))
        qks = ctx.enter_context(tc.tile_pool(name="qks", bufs=2))
        outp = ctx.enter_context(tc.tile_pool(name="outp", bufs=1))
        vts = ctx.enter_context(tc.tile_pool(name="vts", bufs=n_pairs))
        Ep = ctx.enter_context(tc.tile_pool(name="Ep", bufs=6))
        dstp = ctx.enter_context(tc.tile_pool(name="dstp", bufs=2))
        rcp = ctx.enter_context(tc.tile_pool(name="rcp", bufs=2))
        wostp = ctx.enter_context(tc.tile_pool(name="wostp", bufs=3))
        dram = ctx.enter_context(tc.tile_pool(name="dscratch", bufs=2, space="DRAM"))
        # psum: allocation order fixes bank layout; exactly 8 banks
        pqk = ctx.enter_context(tc.tile_pool(name="pqk", bufs=2, space="PSUM"))
        dpsp = ctx.enter_context(tc.tile_pool(name="dpsp", bufs=1, space="PSUM"))
        avpp = ctx.enter_context(tc.tile_pool(name="avpp", bufs=1, space="PSUM"))
        psm = ctx.enter_context(tc.tile_pool(name="psm", bufs=2, space="PSUM"))

        ones32 = consts.tile([128, 32], _f16, tag="ones32")
        nc.vector.memset(ones32, 1.0)
        dummy = consts.tile([128, NT], _f16, tag="dummy")
        nc.vector.memset(dummy, 0.0)
        wq_sb = consts.tile([128, n_pairs, 128], _f16, tag="wq")
        wk_sb = consts.tile([128, n_pairs, 128], _f16, tag="wk")
        wv_sb = consts.tile([128, n_pairs, 128], _f16, tag="wv")

        # ---- input DMAs for pair 0 first, then weights, then the rest ----
        qr_t, kr_t, vr_t = {}, {}, {}

        def issue_raws(pr):
            qr = raws.tile([128, S], _f16, tag="qr", name=f"qr{pr}")
            kr = raws.tile([128, S], _f16, tag="kr", name=f"kr{pr}")
            vr = raws.tile([128, S], _f16, tag="vr", name=f"vr{pr}")
            qr_t[pr], kr_t[pr], vr_t[pr] = qr, kr, vr
            rs = slice(pr * 128, (pr + 1) * 128)
            nc.sync.dma_start(out=qr, in_=q_rows[rs, :])
            nc.sync.dma_start(out=kr, in_=k_rows[rs, :])
            nc.sync.dma_start(out=vr, in_=v_rows[rs, :])

        issue_raws(0)
        nc.sync.dma_start(out=wq_sb, in_=wqT[:, :, :])
        nc.sync.dma_start(out=wk_sb, in_=wkT[:, :, :])
        nc.sync.dma_start(out=wv_sb, in_=wvT[:, :, :])
        issue_raws(1)
        woT_sb = wop.tile([128, n_pairs, e_out], _f16, tag="woT")
        nc.sync.dma_start(out=woT_sb, in_=woT[:, :, :])

        # ---- PE warm-up during the DMA lead-in ----
        for _ in range(4):
            ps = psm.tile([128, NT], _f32, tag="ps")
            nc.tensor.matmul(ps[0:32, :], lhsT=ones32, rhs=dummy,
                             start=True, stop=True)

        q_all_t, k_all_t, vt_t, E_t = {}, {}, {}, {}
        out_all = outp.tile([128, n_pairs, S], _f16, tag="outall")

        # ---------- emission helpers ----------
        def proj_unit(pr, which, st):
            """One [128,512] projection matmul + copy into q_all/k_all."""
            if which == "q":
                src, wt = qr_t[pr], wq_sb
                if pr not in q_all_t:
                    q_all_t[pr] = qks.tile([128, S], _f16, tag="qa", name=f"qa{pr}")
                dst = q_all_t[pr]
            else:
                src, wt = kr_t[pr], wk_sb
                if pr not in k_all_t:
                    k_all_t[pr] = qks.tile([128, S], _f16, tag="ka", name=f"ka{pr}")
                dst = k_all_t[pr]
            ps = psm.tile([128, NT], _f32, tag="ps")
            nc.tensor.matmul(ps, lhsT=wt[:, pr, :],
                             rhs=src[:, st * NT:(st + 1) * NT],
                             start=True, stop=True)
            nc.vector.tensor_copy(out=dst[:, st * NT:(st + 1) * NT], in_=ps)

        def vt_unit(pr, c):
            """vT chunk c: [128t,128i] via transpose-projection matmul."""
            if pr not in vt_t:
                vt_t[pr] = vts.tile([128, C, 128], _f16, tag="vt", name=f"vt{pr}")
            vrc = vr_t[pr].rearrange("p (c t) -> p c t", c=C)
            ps = psm.tile([128, NT], _f32, tag="ps")
            nc.tensor.matmul(ps[:, 0:128], lhsT=vrc[:, c, :],
                             rhs=wv_sb[:, pr, :], start=True, stop=True)
            nc.vector.tensor_copy(out=vt_t[pr][:, c, :], in_=ps[:, 0:128])

        def qk_pair_gen(pr, c):
            """QK chunk c, both heads row-co-executing, + the two exps.

            K=64 matmuls run ~2 cycles/col alone; interleaving the two heads
            (disjoint PE row halves) restores ~1 cycle/col aggregate. Order
            (h0,st0),(h1,st0),(h1,st1),(h0,st1) chains same-stationary pairs.
            """
            ka = k_all_t[pr]
            qa = q_all_t[pr]
            pst = {hh: pqk.tile([128, S], _f32, tag="pqk",
                                name=f"pqk_{pr}_{c}_{hh}") for hh in (0, 1)}

            def mm(hh, st):
                nc.tensor.matmul(
                    pst[hh][:, st * NT:(st + 1) * NT],
                    lhsT=ka[64 * hh:64 * hh + 64, c * 128:(c + 1) * 128],
                    rhs=qa[64 * hh:64 * hh + 64, st * NT:(st + 1) * NT],
                    start=True, stop=True)

            mm(0, 0)
            mm(1, 0)
            mm(1, 1)
            mm(0, 1)
            for hh in (0, 1):
                nc.scalar.activation(out=E_t[(pr, hh)][:, c, :], in_=pst[hh],
                                     func=Exp, scale=0.125)

        dps_t = {}

        def denom_gen(pr, hh, c):
            """4-way col-tiled ones-matmuls (fast co-exec); h0/h1 share the
            bank col-split, so the two heads' groups must NOT interleave in
            time (emit h0's full c-streak, then h1's)."""
            if pr not in dps_t:
                dps_t[pr] = dpsp.tile([128, NT], _f32, tag="dps", name=f"dps{pr}")
            dps = dps_t[pr]
            E = E_t[(pr, hh)]
            for q4 in range(4):
                nc.tensor.matmul(
                    dps[32 * q4:32 * q4 + 32, 256 * hh:256 * hh + 256],
                    lhsT=ones32,
                    rhs=E[:, c, q4 * S4:(q4 + 1) * S4],
                    start=(c == 0), stop=(c == C - 1),
                    tile_position=(0, 32 * q4),
                    skip_group_check=True)

        def roundtrip_scale(pr):
            """denom psum -> DRAM transpose -> recip -> fold into vT."""
            dstage = dstp.tile([128, 2, 256], _f32, tag="dst", name=f"dst{pr}")
            nc.vector.tensor_copy(out=dstage, in_=dps_t[pr].rearrange(
                "p (h x) -> p h x", h=2))
            scr = dram.tile([2048], _f32, tag="scr", name=f"scr{pr}")
            # scr[h*1024 + q4*256 + x] = denom_h[q4*256 + x]
            nc.sync.dma_start(
                out=scr.rearrange("(h a f) -> a h f", h=2, a=4),
                in_=dstage[0:97:32, :, :])
            rcr = rcp.tile([128, 2, C], _f32, tag="rcr", name=f"rcr{pr}")
            rc = rcp.tile([128, 2, C], _f32, tag="rc", name=f"rc{pr}")
            nc.sync.dma_start(
                out=rcr,
                in_=scr.rearrange("(h c p) -> p h c", h=2, p=128))
            nc.vector.reciprocal(out=rc, in_=rcr)
            vt = vt_t[pr]
            for hh in (0, 1):
                nc.vector.tensor_tensor(
                    out=vt[:, :, 64 * hh:64 * hh + 64],
                    in0=vt[:, :, 64 * hh:64 * hh + 64],
                    in1=rc[:, hh, :, None].to_broadcast((128, C, 64)),
                    op=Mult)
            if debug:
                nc.sync.dma_start(out=dbg_rc[pr], in_=rc)
                nc.sync.dma_start(out=dbg_dstage[pr], in_=dstage)
                nc.sync.dma_start(out=dbg_vt[pr], in_=vt)
                if pr == 0:
                    nc.sync.dma_start(out=dbg_E[:, :], in_=E_t[(0, 0)][:, 0, :])

        av_t = {}

        def av_unit(pr, st, c0, nch=2):
            """AV chunks [c0, c0+nch) chained so vt stationary loads hide
            behind the previous co-exec pair's stream."""
            if (pr, st) not in av_t:
                av_t[(pr, st)] = avpp.tile([128, NT], _f32, tag="avp",
                                           name=f"avp{pr}_{st}")
            avp = av_t[(pr, st)]
            for c in range(c0, c0 + nch):
                for hh in (0, 1):
                    nc.tensor.matmul(
                        avp[64 * hh:64 * hh + 64, :],
                        lhsT=vt_t[pr][:, c, 64 * hh:64 * hh + 64],
                        rhs=E_t[(pr, hh)][:, c, st * NT:(st + 1) * NT],
                        start=(c == 0), stop=(c == C - 1),
                        tile_position=(0, 64 * hh),
                        skip_group_check=True)

        def av_copy(pr, st):
            nc.vector.tensor_copy(
                out=out_all[:, pr, st * NT:(st + 1) * NT], in_=av_t[(pr, st)])

        def dummy_mm():
            ps = psm.tile([128, NT], _f32, tag="ps")
            nc.tensor.matmul(ps[0:32, :], lhsT=ones32, rhs=dummy,
                             start=True, stop=True)

        # ---------- the pipeline ----------
        for pr in range(n_pairs):
            E_t[(pr, 0)] = Ep.tile([128, C, S], _f16, tag="E", name=f"E{pr}_0")
            E_t[(pr, 1)] = Ep.tile([128, C, S], _f16, tag="E", name=f"E{pr}_1")

        # initial projections for pair 0 (not hidden behind anything);
        # st0 q+k first so QK chunk 0 can start before st1 lands
        proj_unit(0, "q", 0)
        proj_unit(0, "k", 0)
        proj_unit(0, "q", 1)
        proj_unit(0, "k", 1)

        for pr in range(n_pairs):
            # fill units consumed across this pair's 16 QK gens
            # fill order matters: AV of pair p-1 depends on its denom
            # roundtrip (~3.3us latency from end of pair p-1) -- vt/proj
            # units must come first or the in-order PE queue stalls on AV
            fill = deque()
            if pr > 0:
                # vt units first (vr landed during the previous pair)
                for c in range(C):
                    fill.append(lambda pr=pr, c=c: vt_unit(pr, c))
            if pr + 1 < n_pairs:
                fill.append(lambda pr=pr: proj_unit(pr + 1, "q", 0))
                fill.append(lambda pr=pr: proj_unit(pr + 1, "k", 0))
                fill.append(lambda pr=pr: proj_unit(pr + 1, "q", 1))
                fill.append(lambda pr=pr: proj_unit(pr + 1, "k", 1))
            if pr >= 1:
                for st in range(NS):
                    for c0 in range(C):
                        fill.append(
                            lambda pr=pr, st=st, c0=c0: av_unit(pr - 1, st, c0,
                                                                nch=1))
                    fill.append(lambda pr=pr, st=st: av_copy(pr - 1, st))
            if pr + 2 < n_pairs:
                fill.append(lambda pr=pr: issue_raws(pr + 2))
            if pr == 0:
                # vt last on pair 0: its vr/wv DMAs land after qr/kr, and an
                # early vt unit head-of-line blocks the whole PE queue
                for c in range(C):
                    fill.append(lambda pr=pr, c=c: vt_unit(pr, c))

            total = len(fill)
            done = 0
            for c in range(C):
                qk_pair_gen(pr, c)
                dummy_mm()
                want = (total * (c + 1)) // C
                while done < want:
                    fill.popleft()()
                    done += 1
            while fill:
                fill.popleft()()
            for hh in (0, 1):
                for c in range(C):
                    denom_gen(pr, hh, c)
            roundtrip_scale(pr)

        # tail: while pair 3's denom roundtrip is in flight (PE would idle),
        # open partial Wo accumulations for 6 (st, ec) tiles over fc 0..2 in
        # psum banks freed by the QK/denom pools; their fc3 lands after AV p3.
        pr = n_pairs - 1

        def wo_mm(ops, fc, ec, st, start, stop):
            for mh in (0, 1):
                nc.tensor.matmul(
                    ops[64 * mh:64 * mh + 64, :],
                    lhsT=woT_sb[:, fc,
                                ec * 128 + 64 * mh:ec * 128 + 64 * mh + 64],
                    rhs=out_all[:, fc, st * NT:(st + 1) * NT],
                    start=start, stop=stop,
                    tile_position=(0, 64 * mh),
                    skip_group_check=True)

        part_tiles = {}
        holders = []
        for i in range(2):
            t = pqk.tile([128, S], _f32, tag="pqk", name=f"wopart{i}")
            holders.append(t)
        holders.append(dpsp.tile([128, NT], _f32, tag="dps", name="wopart2"))
        slots = [holders[0][:, 0:NT], holders[0][:, NT:S],
                 holders[1][:, 0:NT], holders[1][:, NT:S],
                 holders[2]]
        part_list = [(st, ec) for st in range(NS) for ec in range(3)][:5]
        for slot, (st, ec) in zip(slots, part_list):
            part_tiles[(st, ec)] = slot
            for fc in range(3):
                wo_mm(slot, fc, ec, st, start=(fc == 0), stop=False)

        for _ in range(20):
            dummy_mm()
        for st in range(NS):
            av_unit(pr, st, 0, nch=C)
            av_copy(pr, st)

        for st in range(NS):
            for ec in range(EC):
                if (st, ec) in part_tiles:
                    ops = part_tiles[(st, ec)]
                    wo_mm(ops, n_pairs - 1, ec, st, start=False, stop=True)
                else:
                    ops = psm.tile([128, NT], _f32, tag="ps")
                    for fc in range(n_pairs):
                        wo_mm(ops, fc, ec, st,
                              start=(fc == 0), stop=(fc == n_pairs - 1))
                wost = wostp.tile([128, NT], _f32, tag="wost")
                nc.vector.tensor_copy(out=wost, in_=ops)
                nc.sync.dma_start(
                    out=out_part[ec * 128:(ec + 1) * 128, st * NT:(st + 1) * NT],
                    in_=wost)

    return nc
